# revision 73
# baseline (speedup 1.0000x reference)
"""Trainium2 Bass kernel for nn_AttentionLayer_47596827574368.

Reference computation (per batch sample b, B=8, C=768, H=W=64, L=4096, Cqk=Cv=96):
  Q = Wq @ X, K = Wk @ X, V = Wv @ X            (X = x[b] as [C, L])
  S = Q^T K   [L, L];  beta = softmax(S, axis=-1)
  O = beta @ V^T      [L, Cv]
  y = gamma * (Wlast @ O^T) + X                 [C, L]

Sharding: data-parallel over batch — one sample per NeuronCore (8 cores).

Device plan (per core):
  X streamed in (chunk, 512-col) pieces; Q/K c-major [96, 4096]; V^T as 32
  blocks [128(k), 97] (col 96 = ones -> softmax denominators ride along in
  the attnV matmul); scores computed transposed S^T[k, q] per 128-k block.
  Softmax uses a global-shift exp (C = est_max + 8 sampled from k-block 0;
  exact per-row max is unnecessary: softmax is shift-invariant and fp32 exp
  has huge dynamic-range headroom). Normalization is applied before the
  final projection; gamma is folded into Wlast on the host.

  The 16.7M-element exp of the score matrix is the ScalarE bottleneck
  (~1ns/element at 1.2GHz, vs the PE's 0.83ns/element of matmul alongside),
  so the PSUM score drain is split: ScalarE runs exact exp on ~2/3 of the
  512-wide units, VectorE runs a one-instruction Schraudolph approximation
  on the rest (uint16(A*s + B) bitcast as bf16, max rel err ~3.3%;
  fp32->uint16 conversion rounds and saturates negatives to 0, so
  deep-negative scores become +0.0). Numerator and denominator use the same
  approximate weights, so softmax stays a proper weighted average; measured
  end-to-end error is 8.6e-3 (budget 2e-2). GPSIMD/Pool cannot read PSUM
  on TRN2, so it only handles the SBUF-side reciprocal broadcasts.

  q columns are processed in chunks [512, 1024, 1024, 1024, 512]: the first
  512 chunk is interleaved with the projection phase, the last 512 chunk
  keeps the final drain short. Scores/exp/attnV run at 512-wide unit
  granularity through 6 single-bank PSUM score slots (up to 6 exps in
  flight across both drain engines); the attnV accumulation trails exp by a
  deep unit lag to ride out exp-latency spikes. Chunk normalization runs
  reciprocal on DVE, broadcast + multiply on Pool off a ScalarE-staged
  SBUF copy. (A Pool-side ones/denom divide simmed 0.2us faster but the
  gpsimd divide op fails the real device compile — TimelineSim does not
  validate op support, so HW-verify any new op type.)
  Each chunk's final projection + residual is spread through the
  next chunk's k loop (residual adds on DVE). The last chunk's tail is the
  critical serial path: it normalizes straight from PSUM in two 256-wide
  halves — both reciprocals issued back-to-back on DVE into disjoint
  slices of ONE rcp tile (separate single-buf ring allocations would
  serialize recip1 behind broadcast0's read), broadcasts on Pool, muls on
  DVE — so the Wlast projection matmuls start ~1.5us earlier than a
  full-width chain; the residual arrives via identity-matmul accumulates
  issued during the normalize latency; the drain is 3 ScalarE + 3 DVE
  bf16 copies feeding three paired DMAs on SP (each group one ScalarE +
  one DVE copy; bf16 halves the tail wire, and the 0.39% step is far
  under the error budget — the host converts back and stitches). Matmuls
  run in float32r (full PE rate). PSUM: proj phase 3+1+2 banks (+2 aux);
  main phase 6 score slots + 2 accumulator banks.

  Note for future tuning: the Tile framework list-schedules per engine by
  readiness (program order is only a tie-break), so "issue X later" code
  motion does NOT delay X — boundary ops like the ostage copy hoist into
  any engine idle slot as soon as their inputs are ready. A 1024-wide
  paired-exp variant (one exp per two 512 units; see kernel_pair.py) cuts
  ~17us of drain-engine busy time but loses ~7us to coarser score-slot
  recycling at chunk boundaries — net worse while the PE, not the drain
  engines, is the bottleneck (~91% busy).

  The last phase4 output block of chunk 3 is issued late (after the last
  chunk's attnV drain, identity-residual + ScalarE-copy form): its four
  matmuls fill the PE gap while the tail normalize chain runs, and its
  ~1.2us DVE add stays out of the tail window. The first of the last
  chunk's four trailing exps runs on DVE (tailpat 'vaaa') so the z-ring
  slots recycle without serializing behind ScalarE's backlog.

  Timeline-sim: 169301 ns/core (prior session: 171303, original baseline:
  198420); measured HW rel err 8.6e-3 (budget 2e-2). The mid-chunk DVE
  exp residues (vset u%8 in {1,4}) came from a joint sweep — {1,4}/{2,4}/
  {0,4} are a flat basin ~300ns below the older {2,5}.
"""

import numpy as np

import concourse.bass as bass
import concourse.tile as tile
import concourse.mybir as mybir
from concourse import bacc
from concourse import bass_utils
from concourse.masks import make_identity

F32 = mybir.dt.float32
F32R = mybir.dt.float32r
BF16 = mybir.dt.bfloat16
U16 = mybir.dt.uint16
AF = mybir.ActivationFunctionType
AX = mybir.AxisListType
OP = mybir.AluOpType

C = 768          # input/output channels
CQ = 96          # qk/v channels
L = 4096         # H*W
KC = C // 128    # 6 contraction chunks
NKB = L // 128   # 32 k blocks
MARGIN = 8.0     # exp shift safety margin

# Schraudolph bf16-bits exp: exp(x) ~ bitcast_bf16(uint16(SCH_A*x + SCH_B)).
# Tuned on-device: fp32->uint16 converts round-to-nearest with saturation.
SCH_A = float(np.float32(128.0 / np.log(2.0)))
SCH_B = 16250.5

CHUNKS = [(0, 512), (512, 1024), (1536, 1024), (2560, 1024), (3584, 512)]
ET_BUFS = 12
S_BUFS = 6      # 512-wide score slots: exp latency tolerance / parallelism

# Schedule knobs (sweepable via sim_trace/sweep scripts):
CFG = {
    "lag": (8, 12, 4),       # attnV unit lag for (chunk0, mid, last)
    "vset": (1, 4),          # u%8 residues DVE takes in mid chunks
    "tailpat": "vaaa",       # engines of the last chunk's last 4 exps
    "bridge_eng": "av",      # bridge exp engines (pool-close race)
    "split_start": False,    # split the first x piece into 256-col halves
    "psum_recip": False,     # recip from PSUM denom row (worse in sim)
    "c0tail": "avav",        # engines of chunk 0's last 4 exps
}


def attnv_lag(ci):
    """attnV trails exp by this many 512-wide units. Chunk 0's drain gates
    the PSUM pool swap and the last chunk's drain gates the tail, so they
    may use shallower lags than the latency-tolerant mid chunks."""
    l0, lm, ll = CFG["lag"]
    if ci == 0:
        return l0
    if ci == len(CHUNKS) - 1:
        return ll
    return lm


def exp_engine(ci, u, n_units):
    """Which engine computes exp for 512-wide unit u (= kb*halves + h) of
    chunk ci. 'a' = ScalarE exact exp; 'v' = DVE Schraudolph approx (~1/3
    of units, validated end-to-end at ~7e-3 rel err).

    GPSIMD/Pool cannot read PSUM on TRN2 (BIR verifier rejects it), so only
    ScalarE and DVE can drain the score tiles. A chunk's first 8 units stay
    on ScalarE (DVE's queue holds the previous chunk's recip/mul then); two
    of the last 4 go to DVE so the trailing exps — which gate the next
    chunk's score slots — don't sit behind ScalarE's backlog."""
    if u >= n_units - 4:
        # trailing units gate the next chunk's score slots (or the tail's
        # z-ring); split them across both engines so neither serializes
        if ci == len(CHUNKS) - 1:
            return CFG.get("tailpat", "aaaa")[u % 4]
        if ci == 0:
            return CFG.get("c0tail", "vava")[u % 4]
        return 'vava'[u % 4]
    if ci == 0:
        # proj phase: mostly ScalarE (it has slack at 512-wide tiles), with
        # a small DVE share so chunk-0's score slots never back up
        m, r, lo = CFG.get("c0v", (4, 1, 4))
        return 'v' if (u % m == r and u >= lo) else 'a'
    if u < 8:
        # alternate from the start: DVE only pays the previous chunk's
        # reciprocal now, so it can take every other early unit
        return CFG.get("head", "vavavava")[u]
    vs = CFG.get(f"vset{ci}") or CFG.get("vset", (2, 5, 7))
    if u % 8 in vs:
        return 'v'
    return 'a'


def pieces(w):
    # split a chunk width into matmul-sized pieces (<=512, >=256 so f32r
    # stays at full rate and no PSUM bank is crossed)
    out = []
    off = 0
    while w - off > 512:
        out.append((off, 512))
        off += 512
    out.append((off, w - off))
    return out


def body(nc, tc, sbuf, x, wqkv_t, wl_t, y, y2):
    # ---- persistent sbuf tiles -----------------------------------------
    # weights first (per-kc pieces so the first proj matmul starts early)
    w_sb = sbuf.tile([128, KC, 3 * CQ], F32R, tag="w")
    w_r = wqkv_t.rearrange("(ko ki) m -> ki ko m", ki=128).bitcast(F32R)
    # DMA issue costs ~650ns of sequencer time apiece, so startup spreads
    # issues across queues: weights on Pool's SWDGE, x on SP — the first
    # w slice (just Wq kc=0) and the first x piece reach the (shared,
    # serial) HWDGE back-to-back instead of ~1.3us apart; the identity
    # setup must come AFTER these issues (its Pool memsets would delay the
    # w SWDGE descriptor generation by ~1.5us -> +0.9us end-to-end)
    nc.gpsimd.dma_start(out=w_sb[:, 0, 0:CQ], in_=w_r[:, 0, 0:CQ])

    x_sb = sbuf.tile([128, KC, L], F32R, tag="x")
    x_r = x.rearrange("(ko ki) l -> ki ko l", ki=128).bitcast(F32R)
    # first piece split in half: the first projection matmuls run on
    # [*, 0:256] and start ~0.35us earlier (half the first-piece wire)
    if CFG.get("split_start", True):
        nc.sync.dma_start(out=x_sb[:, 0, 0:256], in_=x_r[:, 0, 0:256])
        nc.scalar.dma_start(out=w_sb[:, 0, CQ:], in_=w_r[:, 0, CQ:])
        nc.sync.dma_start(out=x_sb[:, 0, 256:512], in_=x_r[:, 0, 256:512])
    elif CFG.get("x0_swdge"):
        # first piece through the software DGE on Pool's queue: ~25ns seq
        # + ~1us fixed vs the HWDGE path's ~0.6+1.8us — lands ~0.8us sooner
        nc.gpsimd.dma_start(out=x_sb[:, 0, 0:512], in_=x_r[:, 0, 0:512])
        nc.scalar.dma_start(out=w_sb[:, 0, CQ:], in_=w_r[:, 0, CQ:])
    else:
        nc.sync.dma_start(out=x_sb[:, 0, 0:512], in_=x_r[:, 0, 0:512])
        nc.scalar.dma_start(out=w_sb[:, 0, CQ:], in_=w_r[:, 0, CQ:])
    # interleave the remaining weight pieces between group-0 x pieces in
    # demand order — weights queued up-front would push the x stream (the
    # projection pacer) back by ~3us
    if CFG.get("merge_w", False):
        # one merged DMA for w2..w5 frees three serial ~625ns HWDGE slots,
        # pulling every group-1/2 x piece earlier (they pace the PE around
        # 10-13us); w1 stays separate so kc=1's matmul isn't gated on the
        # whole merged wire
        nc.sync.dma_start(out=x_sb[:, 1, 0:512], in_=x_r[:, 1, 0:512])
        nc.scalar.dma_start(out=w_sb[:, 1, :], in_=w_r[:, 1, :])
        nc.sync.dma_start(out=x_sb[:, 2, 0:512], in_=x_r[:, 2, 0:512])
        nc.scalar.dma_start(out=w_sb[:, 2:, :], in_=w_r[:, 2:, :])
        for kc in range(3, KC):
            nc.sync.dma_start(out=x_sb[:, kc, 0:512], in_=x_r[:, kc, 0:512])
    else:
        for kc in range(1, KC):
            nc.sync.dma_start(out=x_sb[:, kc, 0:512], in_=x_r[:, kc, 0:512])
            nc.scalar.dma_start(out=w_sb[:, kc, :], in_=w_r[:, kc, :])
    wl_sb = sbuf.tile([CQ, C], F32R, tag="wl")
    # remaining groups in consumption order (wl after group 1 — first
    # needed ~60us in, by phase4 of chunk 0)
    for gp in range(1, 8):
        gs = slice(gp * 512, (gp + 1) * 512)
        for kc in range(KC):
            nc.sync.dma_start(out=x_sb[:, kc, gs], in_=x_r[:, kc, gs])
        if gp == CFG.get("wl_after", 1):
            nc.scalar.dma_start(out=wl_sb, in_=wl_t.bitcast(F32R))

    ident = sbuf.tile([128, 128], F32, tag="ident")
    make_identity(nc, ident)
    ident_bf = sbuf.tile([128, 128], BF16, tag="identbf")
    make_identity(nc, ident_bf)
    # f32r copy of the identity (the residual-add matmul needs an f32r
    # producer; a plain bitcast of the F32 tile fails BIR verification)
    ident_r = sbuf.tile([128, 128], F32R, tag="identr")
    nc.scalar.copy(ident_r, ident)

    q_sb = sbuf.tile([CQ, L], F32R, tag="q")
    k_sb = sbuf.tile([CQ, L], F32R, tag="k")
    v_sb = sbuf.tile([CQ, L], BF16, tag="vbig")
    vt_sb = sbuf.tile([128, NKB, CQ + 1], BF16, tag="vt")
    # ones column (f32r producer required: memset can't write f32r)
    nc.scalar.activation(
        out=vt_sb[:, :, CQ : CQ + 1].rearrange("p a b -> p (a b)"),
        in_=ident[:, 0:NKB],
        func=AF.Copy,
        bias=1.0,
        scale=0.0,
    )

    small = sbuf.tile([128, 16], F32, tag="small")
    m_row = small[:, 8:9]
    neg_c = small[:, 9:10]
    gmax_bc = small[:, 10:11]
    b_eff = small[:, 12:13]       # SCH_A * neg_c + SCH_B  (per partition)
    mt_sb = sbuf.tile([1, 128], F32, tag="rcp")

    attn_sb = sbuf.tile([CQ, L], F32R, tag="vbig", name="attn_sb")
    rcp_bc = sbuf.tile([CQ, 1024], F32, tag="rbc")
    y_r = y.rearrange("(ko ki) l -> ki ko l", ki=128)
    y2_r = y2.rearrange("(ko ki) l -> ki ko l", ki=128)

    def scores_mms(s_ps, kb, c0, w):
        for off, pw in pieces(w):
            nc.tensor.matmul(
                s_ps[:, off : off + pw],
                k_sb[:, kb * 128 : (kb + 1) * 128],
                q_sb[:, c0 + off : c0 + off + pw],
                start=True,
                stop=True,
            )

    def attnv_mm(out_ps, et, kb, off, pw):
        nc.tensor.matmul(
            out_ps[0 : CQ + 1, off : off + pw],
            vt_sb[:, kb, :],
            et[:, 0:pw],
            start=(kb == 0),
            stop=(kb == NKB - 1),
        )

    def exp_tile(ci, u, s_ps, pw, n_units=NKB * 2):
        """exp(s - C) into a bf16 et tile, on the engine exp_engine says."""
        et = sbuf.tile([128, 512], BF16, tag="et", bufs=ET_BUFS,
                       name=f"et_{ci}_{u}")[:, 0:pw]
        eng = exp_engine(ci, u, n_units)
        if eng == 'a':
            nc.scalar.activation(et, s_ps, AF.Exp, bias=neg_c, scale=1.0)
        else:
            nc.vector.tensor_scalar(et.bitcast(U16), s_ps, SCH_A, b_eff,
                                    OP.mult, OP.add)
        return et

    def normalize(ci, out_ps):
        #   attn[:, c0:c0+w] = out_ps[0:96] * (1 / out_ps[96])
        # Mid chunks read straight from PSUM (no staging copy): the
        # accumulator's banks are only needed again two chunks later, and
        # skipping the copy cuts the serial recip -> broadcast -> mul latency
        # at each chunk boundary. Chunk 0 lives in the ps_proj pool whose
        # close barrier gates all of chunk 1, so it stages through SBUF with
        # one fast ScalarE copy to release its banks immediately.
        c0, w = CHUNKS[ci]
        if ci == len(CHUNKS) - 1:
            # tail: DVE is otherwise idle; straight from PSUM is the
            # shortest chain before phase4
            rcp_sb = sbuf.tile([1, 1024], F32, tag="rcp",
                               name=f"rcp_{ci}")[:, 0:w]
            nc.vector.reciprocal(rcp_sb, out_ps[CQ : CQ + 1, 0:w])
            nc.gpsimd.partition_broadcast(rcp_bc[:, 0:w], rcp_sb)
            nc.vector.tensor_mul(attn_sb[:, c0 : c0 + w], out_ps[0:CQ, 0:w],
                                 rcp_bc[:, 0:w])
        else:
            # stage through SBUF (ScalarE) and multiply on Pool: DVE only
            # pays the reciprocal, so its exp share starts ~2us earlier at
            # each chunk boundary — where ScalarE alone can't keep the
            # score slots draining at the PE's pace. Also frees the
            # accumulator's banks early (obig has a single slot).
            # the denominator row rides in the staged copy, so the
            # reciprocal reads SBUF — for chunk 0 that takes it off the
            # ps_proj pool-close path (which gates all of chunk 1)
            ostage = sbuf.tile([CQ + 1, 1024], F32, tag="ostage", bufs=1,
                               name=f"ostage_{ci}")[:, 0:w]
            nc.scalar.copy(ostage, out_ps[0 : CQ + 1, 0:w])
            rcp_sb = sbuf.tile([1, 1024], F32, tag="rcp",
                               name=f"rcp_{ci}")[:, 0:w]
            nc.vector.reciprocal(rcp_sb, ostage[CQ : CQ + 1, :])
            nc.gpsimd.partition_broadcast(rcp_bc[:, 0:w], rcp_sb)
            nc.gpsimd.tensor_mul(attn_sb[:, c0 : c0 + w], ostage[0:CQ, :],
                                 rcp_bc[:, 0:w])

    norm_tiles = {}

    def normalize_half(ci, out_ps, h):
        # Mid-chunk normalize, one 512-wide half at a time. Half 0 is
        # issued at the chunk's end; half 1 from inside the NEXT chunk's
        # k loop, a few units in: both halves are ready at the boundary,
        # so only priority (issue order) decides what the list scheduler
        # hoists into the boundary — deferring half 1 puts the next
        # chunk's first exps ahead of it in the ready-queue tie-breaks,
        # halving the boundary-resident ostage/recip work.
        c0, w = CHUNKS[ci]
        hw2 = w // 2
        hs = slice(h * hw2, (h + 1) * hw2)
        if h == 0:
            norm_tiles[ci] = (
                sbuf.tile([CQ + 1, 1024], F32, tag="ostage", bufs=1,
                          name=f"ostage_{ci}"),
                sbuf.tile([1, 1024], F32, tag="rcp", name=f"rcp_{ci}"),
            )
        ostage, rcp_sb = norm_tiles[ci]
        nc.scalar.copy(ostage[:, hs], out_ps[0 : CQ + 1, hs])
        nc.vector.reciprocal(rcp_sb[:, hs], ostage[CQ : CQ + 1, hs])
        nc.gpsimd.partition_broadcast(rcp_bc[:, hs], rcp_sb[:, hs])
        nc.gpsimd.tensor_mul(attn_sb[:, c0 + h * hw2 : c0 + (h + 1) * hw2],
                             ostage[0:CQ, hs], rcp_bc[:, hs])

    def phase4_unit(ps_pool, ci, oc, spread=True):
        # final projection + residual for one 128-row output chunk
        c0, w = CHUNKS[ci]
        if spread:
            # z halves borrow score slots (exp lookahead briefly 6 -> 4);
            # each half's add fires as soon as its matmul lands, DVE taking
            # one half and Pool the other
            y_sb = sbuf.tile([128, 1024], F32, tag="y", bufs=3,
                             name=f"y_sb_{ci}_{oc}")[:, 0:w]
            for g, (off, pw) in enumerate(pieces(w)):
                z_ps = ps_pool.tile([128, 512], F32, tag="s", bufs=S_BUFS,
                                    name=f"z_ps_{ci}_{oc}_{g}")[:, 0:pw]
                nc.tensor.matmul(
                    z_ps,
                    wl_sb[:, oc * 128 : (oc + 1) * 128],
                    attn_sb[:, c0 + off : c0 + off + pw],
                    start=True,
                    stop=True,
                )
                nc.vector.tensor_add(y_sb[:, off : off + pw], z_ps,
                               x_sb[:, oc, c0 + off : c0 + off + pw].bitcast(F32))
            nc.sync.dma_start(out=y_r[:, oc, slice(c0, c0 + w)], in_=y_sb)
            return
        # late-issued unit: residual via identity-matmul accumulate and a
        # ScalarE copy — its matmuls fill the last chunk's normalize-wait
        # PE gap, and it keeps the ~1.2us DVE add out of the tail window
        y_sb = sbuf.tile([128, 1024], F32, tag="y", bufs=3,
                         name=f"y_sb_{ci}_{oc}")[:, 0:w]
        for g, (off, pw) in enumerate(pieces(w)):
            zp = ps_pool.tile([128, 512], F32, tag="s", bufs=S_BUFS,
                              name=f"z_late_{ci}_{oc}_{g}")[:, 0:pw]
            nc.tensor.matmul(zp, ident_r, x_sb[:, oc, c0 + off : c0 + off + pw],
                             start=True, stop=False)
            nc.tensor.matmul(
                zp,
                wl_sb[:, oc * 128 : (oc + 1) * 128],
                attn_sb[:, c0 + off : c0 + off + pw],
                start=False,
                stop=True,
            )
            nc.scalar.copy(y_sb[:, off : off + pw], zp)
        nc.sync.dma_start(out=y_r[:, oc, slice(c0, c0 + w)], in_=y_sb)

    def phase4_last(ps_pool, ci, out_ps):
        # Last chunk: normalize from PSUM in two 256-wide halves (both
        # reciprocals on DVE up front, broadcasts on Pool, then the muls)
        # so the first projection matmuls start ~1us earlier than a
        # full-width chain; residual via identity-matmul accumulates
        # issued during the normalize latency; PSUM->SBUF bf16 copies
        # split 3 ScalarE / 3 DVE feeding three paired DMAs on SP (each
        # group one ScalarE + one DVE copy, so the last DMA issues as
        # early as either engine allows).
        c0, w = CHUNKS[ci]
        hw = w // 2
        zs = []
        for oc in range(KC):
            z_ps = ps_pool.tile([128, 512], F32, tag="s", bufs=S_BUFS,
                                name=f"z_ps_{ci}_{oc}")[:, 0:w]
            nc.tensor.matmul(
                z_ps,
                ident_r,
                x_sb[:, oc, c0 : c0 + w],
                start=True,
                stop=False,
            )
            zs.append(z_ps)
        # one rcp tile sliced per half: separate ring allocations would
        # serialize recip1 behind bcast0's read of the single-buf slot
        rcp2 = sbuf.tile([1, 1024], F32, tag="rcp", name=f"rcp_{ci}")
        for h in range(2):
            hs = slice(h * hw, (h + 1) * hw)
            nc.vector.reciprocal(rcp2[:, hs], out_ps[CQ : CQ + 1, hs])
            nc.gpsimd.partition_broadcast(rcp_bc[:, hs], rcp2[:, hs])
        for h in range(2):
            hs = slice(h * hw, (h + 1) * hw)
            nc.vector.tensor_mul(attn_sb[:, c0 + h * hw : c0 + (h + 1) * hw],
                                 out_ps[0:CQ, hs], rcp_bc[:, hs])
            for oc in range(KC):
                nc.tensor.matmul(
                    zs[oc][:, hs],
                    wl_sb[:, oc * 128 : (oc + 1) * 128],
                    attn_sb[:, c0 + h * hw : c0 + (h + 1) * hw],
                    start=False,
                    stop=(h == 1),
                )
        ysg = [sbuf.tile([128, 2, 512], BF16, tag=f"ylast{g}", bufs=1,
                         name=f"y_last_{g}") for g in range(3)]
        for g in range(3):
            nc.scalar.copy(ysg[g][:, 0, :], zs[2 * g])
            nc.vector.tensor_copy(ysg[g][:, 1, :], zs[2 * g + 1])
        # SP queue only: a ScalarE-issued DMA would block ScalarE's in-order
        # queue and delay the remaining copies
        for g in range(3):
            nc.sync.dma_start(out=y2_r[:, 2 * g : 2 * g + 2, :], in_=ysg[g])

    # ---- phase 1 + attention chunk 0 (512 wide), interleaved ------------
    # projections run in 512-column groups; as each group's K/V land, the
    # corresponding k-blocks of chunk 0 are scored/exp'd/accumulated.
    with (
        tc.tile_pool(name="ps_proj", bufs=1, space="PSUM") as ps_proj,
        tc.tile_pool(name="ps_aux", bufs=2, space="PSUM") as ps_aux,
    ):
        out0_ps = ps_proj.tile([128, 512], F32, tag="o0", name="out0_ps")
        # PE p-state warmup: the clock runs at half rate until 3us of
        # CONTINUOUS busy, and the first x piece only lands ~3.9us in.
        # Dummy identity transposes (no readers, recycled ps_aux ring)
        # keep the PE busy from ~0.3us so the ramp completes before the
        # first projection matmul — which then runs at the full 2.4GHz.
        for wu in range(CFG.get("warmup", 0)):
            wu_ps = ps_aux.tile([128, 128], F32, tag="sm", name=f"wu_{wu}")
            nc.tensor.transpose(wu_ps, ident, ident)
        pend_attnv = []  # attnV lag FIFO so PE never waits on exp in-order
        for gp in range(8):
            gs = slice(gp * 512, (gp + 1) * 512)
            tiles = [
                ps_proj.tile([CQ, 512], F32, tag=f"proj{t}", name=f"p_ps_{t}_{gp}")
                for t in range(3)
            ]
            for kc in range(KC):
                for t in range(3):
                    if gp == 0 and kc == 0 and CFG.get("split_start", True):
                        # x arrives in two 256-col halves; start on the first
                        for ho in (0, 256):
                            nc.tensor.matmul(
                                tiles[t][:, ho : ho + 256],
                                w_sb[:, 0, t * CQ : (t + 1) * CQ],
                                x_sb[:, 0, ho : ho + 256],
                                start=True,
                                stop=False,
                                skip_group_check=True,
                            )
                        continue
                    nc.tensor.matmul(
                        tiles[t],
                        w_sb[:, kc, t * CQ : (t + 1) * CQ],
                        x_sb[:, kc, gs],
                        start=(kc == 0),
                        stop=(kc == KC - 1),
                        skip_group_check=(gp == 0),
                    )
            for t, dst in ((0, q_sb), (1, k_sb), (2, v_sb)):
                if t == 1:
                    nc.vector.tensor_copy(dst[:, gs], tiles[t])
                else:
                    nc.scalar.copy(dst[:, gs], tiles[t])

            # chunk-0 attention for this group's 4 k-blocks
            for kb in range(4 * gp, 4 * gp + 4):
                s_ps = ps_proj.tile([128, 512], F32, tag="s0", bufs=2,
                                    name=f"s_ps_0_{kb}")
                scores_mms(s_ps, kb, 0, 512)
                if kb == 0:
                    # shift estimate from these 65k scores (statistically
                    # ample for a shift that merely has to land within
                    # ~+-80 of the true max)
                    nc.vector.reduce_max(m_row, s_ps, axis=AX.X)
                    mt_ps = ps_aux.tile([1, 128], F32, tag="sm")
                    nc.tensor.transpose(mt_ps, m_row, ident)
                    nc.vector.tensor_copy(mt_sb[:, 0:128], mt_ps)
                    nc.vector.reduce_max(small[0:1, 11:12], mt_sb[:, 0:128],
                                         axis=AX.X)
                    nc.gpsimd.partition_broadcast(gmax_bc, small[0:1, 11:12])
                    # neg_c = -(gmax + MARGIN)
                    nc.scalar.activation(neg_c, gmax_bc, AF.Copy,
                                         bias=-MARGIN, scale=-1.0)
                    # b_eff = SCH_A * neg_c + SCH_B (for the approx engines)
                    nc.vector.tensor_scalar(b_eff, neg_c, SCH_A, SCH_B,
                                            OP.mult, OP.add)
                et = exp_tile(0, kb, s_ps, 512, n_units=NKB)
                if len(pend_attnv) >= attnv_lag(0):
                    attnv_mm(out0_ps, *pend_attnv.pop(0))
                pend_attnv.append((et, kb, 0, 512))
            # V -> V^T transposes for this group's 4 l-blocks (the last
            # group's copies optionally on ScalarE so DVE is free for the
            # bridge exps that gate the pool swap)
            for lb in range(4 * gp, 4 * gp + 4):
                t_ps = ps_aux.tile([128, CQ], BF16, tag="sm", name=f"t_ps_{lb}")
                nc.tensor.transpose(
                    t_ps, v_sb[:, lb * 128 : (lb + 1) * 128], ident_bf[0:CQ, 0:CQ]
                )
                if gp == 7 and CFG.get("vt7_scalar"):
                    nc.scalar.copy(vt_sb[:, lb, 0:CQ], t_ps)
                else:
                    nc.vector.tensor_copy(vt_sb[:, lb, 0:CQ], t_ps)

        for pa in pend_attnv:
            attnv_mm(out0_ps, *pa)
        # chunk-0 normalize first: its ScalarE staging copy releases the
        # out0 banks so the pool-close barrier (gating all of chunk 1) isn't
        # stuck behind the bridge exps
        normalize(0, out0_ps)
        # bridge: score+exp chunk-1's k-block 0 in this pool's slots so
        # ScalarE never idles across the PSUM pool swap
        bridge_units = []
        for bu in range(2 * CFG.get("bridge_kb", 1)):
            kb, h = bu // 2, bu % 2
            sb_ps = ps_proj.tile([128, 512], F32, tag="s0", bufs=2,
                                 name=f"sb_ps_{bu}")
            nc.tensor.matmul(
                sb_ps, k_sb[:, kb * 128 : (kb + 1) * 128],
                q_sb[:, 512 + h * 512 : 512 + (h + 1) * 512],
                start=True, stop=True,
            )
            bet = sbuf.tile([128, 512], BF16, tag="et", bufs=ET_BUFS,
                            name=f"et_1_0_{bu}")
            # engine choice: the pool close (gating all of chunk 1) waits on
            # these exps' PSUM reads, racing the other engine's backlog
            if CFG.get("bridge_eng", "vv")[bu % len(CFG.get("bridge_eng", "vv"))] == 'v':
                nc.vector.tensor_scalar(bet.bitcast(U16), sb_ps, SCH_A, b_eff,
                                        OP.mult, OP.add)
            else:
                nc.scalar.activation(bet, sb_ps, AF.Exp, bias=neg_c, scale=1.0)
            bridge_units.append((bet, kb, h * 512, 512))

    # ---- attention chunks 1..4 ------------------------------------------
    with tc.tile_pool(name="ps_attn", bufs=1, space="PSUM") as ps_attn:
        prev_ps = [None]   # previous chunk's accumulator awaiting half-1
        for ci in range(1, len(CHUNKS)):
            c0, w = CHUNKS[ci]
            out_ps = ps_attn.tile(
                [128, 1024], F32, tag="obig", bufs=1, name=f"out_ps_{ci}"
            )
            # attnV trails exp by ATTNV_LAG 512-wide units; 4 s_ps slots let
            # up to 4 exps run concurrently across ScalarE/DVE/Pool
            pend = list(bridge_units) if ci == 1 else []
            nh = len(pieces(w))
            for kb in range(CFG.get("bridge_kb", 1) if ci == 1 else 0, NKB):
                for h, (off, pw) in enumerate(pieces(w)):
                    u = kb * nh + h
                    s_ps = ps_attn.tile(
                        [128, 512], F32, tag="s", bufs=S_BUFS,
                        name=f"s_ps_{ci}_{u}"
                    )[:, 0:pw]
                    nc.tensor.matmul(
                        s_ps,
                        k_sb[:, kb * 128 : (kb + 1) * 128],
                        q_sb[:, c0 + off : c0 + off + pw],
                        start=True,
                        stop=True,
                    )
                    et = exp_tile(ci, u, s_ps, pw, n_units=NKB * nh)
                    if len(pend) >= attnv_lag(ci):
                        attnv_mm(out_ps, *pend.pop(0))
                    pend.append((et, kb, off, pw))
                    if u == CFG.get("h1_at", 3) and ci >= 2 and \
                            CFG.get("split_norm", False):
                        normalize_half(ci - 1, prev_ps[0], 1)
                # spread the previous chunk's phase 4 through this chunk's
                # k loop, starting at kb=8 so the previous chunk's normalize
                # chain (which the z matmuls depend on) has finished — PE is
                # in-order, so an early-enqueued z matmul would stall scores
                last = ci == len(CHUNKS) - 1
                sp0, step = CFG.get("sp_last", (16, 3)) if last else CFG.get("sp_mid", (12, 3))
                # all spread units must fit inside this chunk's kb range — a
                # unit past kb=31 would silently drop an output block
                nsp = KC - 1 if (last and CFG.get("late6", True)) else KC
                assert sp0 + step * (nsp - 1) < NKB
                if (kb - sp0) % step == 0 and sp0 <= kb < sp0 + step * nsp:
                    phase4_unit(ps_attn, ci - 1, (kb - sp0) // step)
            for pe in pend:
                attnv_mm(out_ps, *pe)
            if ci == len(CHUNKS) - 1 and CFG.get("late6", True):
                # chunk-3's last output block, issued after the drain: its
                # matmuls fill the PE gap while the tail normalize runs
                phase4_unit(ps_attn, ci - 1, KC - 1, spread=False)
            if ci < len(CHUNKS) - 1:
                if CFG.get("split_norm", False):
                    normalize_half(ci, out_ps, 0)
                    prev_ps[0] = out_ps
                else:
                    normalize(ci, out_ps)

        # last chunk's normalize halves + phase 4
        phase4_last(ps_attn, len(CHUNKS) - 1, out_ps)


def build(loop_iters=1):
    nc = bacc.Bacc("TRN2", target_bir_lowering=False, debug=False, num_devices=8)
    x = nc.dram_tensor("x", [C, L], F32, kind="ExternalInput").ap()
    wqkv_t = nc.dram_tensor("wqkv_t", [C, 3 * CQ], F32, kind="ExternalInput").ap()
    wl_t = nc.dram_tensor("wl_t", [CQ, C], F32, kind="ExternalInput").ap()
    y = nc.dram_tensor("y", [C, L], F32, kind="ExternalOutput").ap()
    # last q-chunk's output in bf16: halves the tail's DMA wire time; the
    # 0.39% bf16 step is well under the error budget (host converts back)
    y2 = nc.dram_tensor("y2", [C, CHUNKS[-1][1]], mybir.dt.bfloat16,
                        kind="ExternalOutput").ap()

    with tile.TileContext(nc) as tc:
        with tc.tile_pool(name="sbuf", bufs=1) as sbuf:
            if loop_iters > 1:
                engines = (
                    mybir.EngineType.PE,
                    mybir.EngineType.Activation,
                    mybir.EngineType.DVE,
                    mybir.EngineType.Pool,
                    mybir.EngineType.SP,
                )
                with tc.For_i(0, loop_iters, hint_engines=engines):
                    body(nc, tc, sbuf, x, wqkv_t, wl_t, y, y2)
            else:
                body(nc, tc, sbuf, x, wqkv_t, wl_t, y, y2)

    nc.compile()
    return nc


_cached_nc = None


def kernel(x, Wq, Wk, Wv, Wlast, gamma):
    global _cached_nc
    x = np.ascontiguousarray(np.asarray(x, dtype=np.float32))
    B = x.shape[0]
    assert B == 8 and x.shape[1:] == (C, 64, 64)
    wqkv_t = np.ascontiguousarray(
        np.concatenate([Wq, Wk, Wv], axis=0).T.astype(np.float32)
    )
    wl_t = np.ascontiguousarray(
        (np.asarray(Wlast, np.float32) * np.float32(np.asarray(gamma)[0])).T
    )

    if _cached_nc is None:
        _cached_nc = build()
    nc = _cached_nc

    in_maps = [
        {
            "x": np.ascontiguousarray(x[b].reshape(C, L)),
            "wqkv_t": wqkv_t,
            "wl_t": wl_t,
        }
        for b in range(B)
    ]
    res = bass_utils.run_bass_kernel_spmd(nc, in_maps, core_ids=list(range(B)))
    lw = CHUNKS[-1][1]
    outs = []
    for b in range(B):
        yb = np.array(res.results[b]["y"]).reshape(C, L)
        yb[:, L - lw:] = res.results[b]["y2"].astype(np.float32)
        outs.append(yb.reshape(C, 64, 64))
    return np.stack(outs).astype(np.float32)



# revision 76
# speedup vs baseline: 1.0012x; 1.0012x over previous
"""Trainium2 Bass kernel for nn_AttentionLayer_47596827574368.

Reference computation (per batch sample b, B=8, C=768, H=W=64, L=4096, Cqk=Cv=96):
  Q = Wq @ X, K = Wk @ X, V = Wv @ X            (X = x[b] as [C, L])
  S = Q^T K   [L, L];  beta = softmax(S, axis=-1)
  O = beta @ V^T      [L, Cv]
  y = gamma * (Wlast @ O^T) + X                 [C, L]

Sharding: data-parallel over batch — one sample per NeuronCore (8 cores).

Device plan (per core):
  X streamed in (chunk, 512-col) pieces; Q/K c-major [96, 4096]; V^T as 32
  blocks [128(k), 97] (col 96 = ones -> softmax denominators ride along in
  the attnV matmul); scores computed transposed S^T[k, q] per 128-k block.
  Softmax uses a global-shift exp (C = est_max + 8 sampled from k-block 0;
  exact per-row max is unnecessary: softmax is shift-invariant and fp32 exp
  has huge dynamic-range headroom). Normalization is applied before the
  final projection; gamma is folded into Wlast on the host.

  The 16.7M-element exp of the score matrix is the ScalarE bottleneck
  (~1ns/element at 1.2GHz, vs the PE's 0.83ns/element of matmul alongside),
  so the PSUM score drain is split: ScalarE runs exact exp on ~2/3 of the
  512-wide units, VectorE runs a one-instruction Schraudolph approximation
  on the rest (uint16(A*s + B) bitcast as bf16, max rel err ~3.3%;
  fp32->uint16 conversion rounds and saturates negatives to 0, so
  deep-negative scores become +0.0). Numerator and denominator use the same
  approximate weights, so softmax stays a proper weighted average; measured
  end-to-end error is 8.6e-3 (budget 2e-2). GPSIMD/Pool cannot read PSUM
  on TRN2, so it only handles the SBUF-side reciprocal broadcasts.

  q columns are processed in chunks [512, 1024, 1024, 1024, 512]: the first
  512 chunk is interleaved with the projection phase, the last 512 chunk
  keeps the final drain short. Scores/exp/attnV run at 512-wide unit
  granularity through 6 single-bank PSUM score slots (up to 6 exps in
  flight across both drain engines); the attnV accumulation trails exp by a
  deep unit lag to ride out exp-latency spikes. Chunk normalization runs
  reciprocal on DVE, broadcast + multiply on Pool off a ScalarE-staged
  SBUF copy. (A Pool-side ones/denom divide simmed 0.2us faster but the
  gpsimd divide op fails the real device compile — TimelineSim does not
  validate op support, so HW-verify any new op type.)
  Each chunk's final projection + residual is spread through the
  next chunk's k loop (residual adds on DVE). The last chunk's tail is the
  critical serial path: it normalizes straight from PSUM in two 256-wide
  halves — both reciprocals issued back-to-back on DVE into disjoint
  slices of ONE rcp tile (separate single-buf ring allocations would
  serialize recip1 behind broadcast0's read), broadcasts on Pool, muls on
  DVE — so the Wlast projection matmuls start ~1.5us earlier than a
  full-width chain; the residual arrives via identity-matmul accumulates
  issued during the normalize latency; the drain is 3 ScalarE + 3 DVE
  bf16 copies feeding three paired DMAs on SP (each group one ScalarE +
  one DVE copy; bf16 halves the tail wire, and the 0.39% step is far
  under the error budget — the host converts back and stitches). Matmuls
  run in float32r (full PE rate). PSUM: proj phase 3+1+2 banks (+2 aux);
  main phase 6 score slots + 2 accumulator banks.

  Note for future tuning: the Tile framework list-schedules per engine by
  readiness (program order is only a tie-break), so "issue X later" code
  motion does NOT delay X — boundary ops like the ostage copy hoist into
  any engine idle slot as soon as their inputs are ready. A 1024-wide
  paired-exp variant (one exp per two 512 units; see kernel_pair.py) cuts
  ~17us of drain-engine busy time but loses ~7us to coarser score-slot
  recycling at chunk boundaries — net worse while the PE, not the drain
  engines, is the bottleneck (~91% busy).

  The last phase4 output block of chunk 3 is issued late (after the last
  chunk's attnV drain, identity-residual + ScalarE-copy form): its four
  matmuls fill the PE gap while the tail normalize chain runs, and its
  ~1.2us DVE add stays out of the tail window. The first of the last
  chunk's four trailing exps runs on DVE (tailpat 'vaaa') so the z-ring
  slots recycle without serializing behind ScalarE's backlog.

  Timeline-sim: 169094 ns/core (prior session: 171303, original baseline:
  198420); measured HW rel err 8.5e-3 (budget 2e-2). The exp engine
  schedules (vset {1,4}, midtail 'aava', tailpat 'vaaa', c0tail 'avav')
  came from exhaustive joint sweeps — re-sweep them all after ANY
  structural change; their optima shift and stale settings cost ~500ns.
"""

import numpy as np

import concourse.bass as bass
import concourse.tile as tile
import concourse.mybir as mybir
from concourse import bacc
from concourse import bass_utils
from concourse.masks import make_identity

F32 = mybir.dt.float32
F32R = mybir.dt.float32r
BF16 = mybir.dt.bfloat16
U16 = mybir.dt.uint16
AF = mybir.ActivationFunctionType
AX = mybir.AxisListType
OP = mybir.AluOpType

C = 768          # input/output channels
CQ = 96          # qk/v channels
L = 4096         # H*W
KC = C // 128    # 6 contraction chunks
NKB = L // 128   # 32 k blocks
MARGIN = 8.0     # exp shift safety margin

# Schraudolph bf16-bits exp: exp(x) ~ bitcast_bf16(uint16(SCH_A*x + SCH_B)).
# Tuned on-device: fp32->uint16 converts round-to-nearest with saturation.
SCH_A = float(np.float32(128.0 / np.log(2.0)))
SCH_B = 16250.5

CHUNKS = [(0, 512), (512, 1024), (1536, 1024), (2560, 1024), (3584, 512)]
ET_BUFS = 12
S_BUFS = 6      # 512-wide score slots: exp latency tolerance / parallelism

# Schedule knobs (sweepable via sim_trace/sweep scripts):
CFG = {
    "lag": (8, 12, 4),       # attnV unit lag for (chunk0, mid, last)
    "vset": (1, 4),          # u%8 residues DVE takes in mid chunks
    "tailpat": "vaaa",       # engines of the last chunk's last 4 exps
    "bridge_eng": "av",      # bridge exp engines (pool-close race)
    "split_start": False,    # split the first x piece into 256-col halves
    "psum_recip": False,     # recip from PSUM denom row (worse in sim)
    "c0tail": "avav",        # engines of chunk 0's last 4 exps
    "midtail": "aava",       # engines of mid chunks' last 4 exps
}


def attnv_lag(ci):
    """attnV trails exp by this many 512-wide units. Chunk 0's drain gates
    the PSUM pool swap and the last chunk's drain gates the tail, so they
    may use shallower lags than the latency-tolerant mid chunks."""
    l0, lm, ll = CFG["lag"]
    if ci == 0:
        return l0
    if ci == len(CHUNKS) - 1:
        return ll
    return lm


def exp_engine(ci, u, n_units):
    """Which engine computes exp for 512-wide unit u (= kb*halves + h) of
    chunk ci. 'a' = ScalarE exact exp; 'v' = DVE Schraudolph approx (~1/3
    of units, validated end-to-end at ~7e-3 rel err).

    GPSIMD/Pool cannot read PSUM on TRN2 (BIR verifier rejects it), so only
    ScalarE and DVE can drain the score tiles. A chunk's first 8 units stay
    on ScalarE (DVE's queue holds the previous chunk's recip/mul then); two
    of the last 4 go to DVE so the trailing exps — which gate the next
    chunk's score slots — don't sit behind ScalarE's backlog."""
    if u >= n_units - 4:
        # trailing units gate the next chunk's score slots (or the tail's
        # z-ring); split them across both engines so neither serializes
        if ci == len(CHUNKS) - 1:
            return CFG.get("tailpat", "aaaa")[u % 4]
        if ci == 0:
            return CFG.get("c0tail", "vava")[u % 4]
        return CFG.get("midtail", "vava")[u % 4]
    if ci == 0:
        # proj phase: mostly ScalarE (it has slack at 512-wide tiles), with
        # a small DVE share so chunk-0's score slots never back up
        m, r, lo = CFG.get("c0v", (4, 1, 4))
        return 'v' if (u % m == r and u >= lo) else 'a'
    if u < 8:
        # alternate from the start: DVE only pays the previous chunk's
        # reciprocal now, so it can take every other early unit
        return CFG.get("head", "vavavava")[u]
    vs = CFG.get(f"vset{ci}") or CFG.get("vset", (2, 5, 7))
    if u % 8 in vs:
        return 'v'
    return 'a'


def pieces(w):
    # split a chunk width into matmul-sized pieces (<=512, >=256 so f32r
    # stays at full rate and no PSUM bank is crossed)
    out = []
    off = 0
    while w - off > 512:
        out.append((off, 512))
        off += 512
    out.append((off, w - off))
    return out


def body(nc, tc, sbuf, x, wqkv_t, wl_t, y, y2):
    # ---- persistent sbuf tiles -----------------------------------------
    # weights first (per-kc pieces so the first proj matmul starts early)
    w_sb = sbuf.tile([128, KC, 3 * CQ], F32R, tag="w")
    w_r = wqkv_t.rearrange("(ko ki) m -> ki ko m", ki=128).bitcast(F32R)
    # DMA issue costs ~650ns of sequencer time apiece, so startup spreads
    # issues across queues: weights on Pool's SWDGE, x on SP — the first
    # w slice (just Wq kc=0) and the first x piece reach the (shared,
    # serial) HWDGE back-to-back instead of ~1.3us apart; the identity
    # setup must come AFTER these issues (its Pool memsets would delay the
    # w SWDGE descriptor generation by ~1.5us -> +0.9us end-to-end)
    nc.gpsimd.dma_start(out=w_sb[:, 0, 0:CQ], in_=w_r[:, 0, 0:CQ])

    x_sb = sbuf.tile([128, KC, L], F32R, tag="x")
    x_r = x.rearrange("(ko ki) l -> ki ko l", ki=128).bitcast(F32R)
    # first piece split in half: the first projection matmuls run on
    # [*, 0:256] and start ~0.35us earlier (half the first-piece wire)
    if CFG.get("split_start", True):
        nc.sync.dma_start(out=x_sb[:, 0, 0:256], in_=x_r[:, 0, 0:256])
        nc.scalar.dma_start(out=w_sb[:, 0, CQ:], in_=w_r[:, 0, CQ:])
        nc.sync.dma_start(out=x_sb[:, 0, 256:512], in_=x_r[:, 0, 256:512])
    elif CFG.get("x0_swdge"):
        # first piece through the software DGE on Pool's queue: ~25ns seq
        # + ~1us fixed vs the HWDGE path's ~0.6+1.8us — lands ~0.8us sooner
        nc.gpsimd.dma_start(out=x_sb[:, 0, 0:512], in_=x_r[:, 0, 0:512])
        nc.scalar.dma_start(out=w_sb[:, 0, CQ:], in_=w_r[:, 0, CQ:])
    else:
        nc.sync.dma_start(out=x_sb[:, 0, 0:512], in_=x_r[:, 0, 0:512])
        nc.scalar.dma_start(out=w_sb[:, 0, CQ:], in_=w_r[:, 0, CQ:])
    # interleave the remaining weight pieces between group-0 x pieces in
    # demand order — weights queued up-front would push the x stream (the
    # projection pacer) back by ~3us
    if CFG.get("merge_w", False):
        # one merged DMA for w2..w5 frees three serial ~625ns HWDGE slots,
        # pulling every group-1/2 x piece earlier (they pace the PE around
        # 10-13us); w1 stays separate so kc=1's matmul isn't gated on the
        # whole merged wire
        nc.sync.dma_start(out=x_sb[:, 1, 0:512], in_=x_r[:, 1, 0:512])
        nc.scalar.dma_start(out=w_sb[:, 1, :], in_=w_r[:, 1, :])
        nc.sync.dma_start(out=x_sb[:, 2, 0:512], in_=x_r[:, 2, 0:512])
        nc.scalar.dma_start(out=w_sb[:, 2:, :], in_=w_r[:, 2:, :])
        for kc in range(3, KC):
            nc.sync.dma_start(out=x_sb[:, kc, 0:512], in_=x_r[:, kc, 0:512])
    else:
        for kc in range(1, KC):
            nc.sync.dma_start(out=x_sb[:, kc, 0:512], in_=x_r[:, kc, 0:512])
            nc.scalar.dma_start(out=w_sb[:, kc, :], in_=w_r[:, kc, :])
    wl_sb = sbuf.tile([CQ, C], F32R, tag="wl")
    # remaining groups in consumption order (wl after group 1 — first
    # needed ~60us in, by phase4 of chunk 0)
    for gp in range(1, 8):
        gs = slice(gp * 512, (gp + 1) * 512)
        for kc in range(KC):
            nc.sync.dma_start(out=x_sb[:, kc, gs], in_=x_r[:, kc, gs])
        if gp == CFG.get("wl_after", 1):
            nc.scalar.dma_start(out=wl_sb, in_=wl_t.bitcast(F32R))

    ident = sbuf.tile([128, 128], F32, tag="ident")
    make_identity(nc, ident)
    ident_bf = sbuf.tile([128, 128], BF16, tag="identbf")
    make_identity(nc, ident_bf)
    # f32r copy of the identity (the residual-add matmul needs an f32r
    # producer; a plain bitcast of the F32 tile fails BIR verification)
    ident_r = sbuf.tile([128, 128], F32R, tag="identr")
    nc.scalar.copy(ident_r, ident)

    q_sb = sbuf.tile([CQ, L], F32R, tag="q")
    k_sb = sbuf.tile([CQ, L], F32R, tag="k")
    v_sb = sbuf.tile([CQ, L], BF16, tag="vbig")
    vt_sb = sbuf.tile([128, NKB, CQ + 1], BF16, tag="vt")
    # ones column (f32r producer required: memset can't write f32r)
    nc.scalar.activation(
        out=vt_sb[:, :, CQ : CQ + 1].rearrange("p a b -> p (a b)"),
        in_=ident[:, 0:NKB],
        func=AF.Copy,
        bias=1.0,
        scale=0.0,
    )

    small = sbuf.tile([128, 16], F32, tag="small")
    m_row = small[:, 8:9]
    neg_c = small[:, 9:10]
    gmax_bc = small[:, 10:11]
    b_eff = small[:, 12:13]       # SCH_A * neg_c + SCH_B  (per partition)
    mt_sb = sbuf.tile([1, 128], F32, tag="rcp")

    attn_sb = sbuf.tile([CQ, L], F32R, tag="vbig", name="attn_sb")
    rcp_bc = sbuf.tile([CQ, 1024], F32, tag="rbc")
    y_r = y.rearrange("(ko ki) l -> ki ko l", ki=128)
    y2_r = y2.rearrange("(ko ki) l -> ki ko l", ki=128)

    def scores_mms(s_ps, kb, c0, w):
        for off, pw in pieces(w):
            nc.tensor.matmul(
                s_ps[:, off : off + pw],
                k_sb[:, kb * 128 : (kb + 1) * 128],
                q_sb[:, c0 + off : c0 + off + pw],
                start=True,
                stop=True,
            )

    def attnv_mm(out_ps, et, kb, off, pw):
        nc.tensor.matmul(
            out_ps[0 : CQ + 1, off : off + pw],
            vt_sb[:, kb, :],
            et[:, 0:pw],
            start=(kb == 0),
            stop=(kb == NKB - 1),
        )

    def exp_tile(ci, u, s_ps, pw, n_units=NKB * 2):
        """exp(s - C) into a bf16 et tile, on the engine exp_engine says."""
        et = sbuf.tile([128, 512], BF16, tag="et", bufs=ET_BUFS,
                       name=f"et_{ci}_{u}")[:, 0:pw]
        eng = exp_engine(ci, u, n_units)
        if eng == 'a':
            nc.scalar.activation(et, s_ps, AF.Exp, bias=neg_c, scale=1.0)
        else:
            nc.vector.tensor_scalar(et.bitcast(U16), s_ps, SCH_A, b_eff,
                                    OP.mult, OP.add)
        return et

    def normalize(ci, out_ps):
        #   attn[:, c0:c0+w] = out_ps[0:96] * (1 / out_ps[96])
        # Mid chunks read straight from PSUM (no staging copy): the
        # accumulator's banks are only needed again two chunks later, and
        # skipping the copy cuts the serial recip -> broadcast -> mul latency
        # at each chunk boundary. Chunk 0 lives in the ps_proj pool whose
        # close barrier gates all of chunk 1, so it stages through SBUF with
        # one fast ScalarE copy to release its banks immediately.
        c0, w = CHUNKS[ci]
        if ci == len(CHUNKS) - 1:
            # tail: DVE is otherwise idle; straight from PSUM is the
            # shortest chain before phase4
            rcp_sb = sbuf.tile([1, 1024], F32, tag="rcp",
                               name=f"rcp_{ci}")[:, 0:w]
            nc.vector.reciprocal(rcp_sb, out_ps[CQ : CQ + 1, 0:w])
            nc.gpsimd.partition_broadcast(rcp_bc[:, 0:w], rcp_sb)
            nc.vector.tensor_mul(attn_sb[:, c0 : c0 + w], out_ps[0:CQ, 0:w],
                                 rcp_bc[:, 0:w])
        else:
            # stage through SBUF (ScalarE) and multiply on Pool: DVE only
            # pays the reciprocal, so its exp share starts ~2us earlier at
            # each chunk boundary — where ScalarE alone can't keep the
            # score slots draining at the PE's pace. Also frees the
            # accumulator's banks early (obig has a single slot).
            # the denominator row rides in the staged copy, so the
            # reciprocal reads SBUF — for chunk 0 that takes it off the
            # ps_proj pool-close path (which gates all of chunk 1)
            ostage = sbuf.tile([CQ + 1, 1024], F32, tag="ostage", bufs=1,
                               name=f"ostage_{ci}")[:, 0:w]
            nc.scalar.copy(ostage, out_ps[0 : CQ + 1, 0:w])
            rcp_sb = sbuf.tile([1, 1024], F32, tag="rcp",
                               name=f"rcp_{ci}")[:, 0:w]
            nc.vector.reciprocal(rcp_sb, ostage[CQ : CQ + 1, :])
            nc.gpsimd.partition_broadcast(rcp_bc[:, 0:w], rcp_sb)
            nc.gpsimd.tensor_mul(attn_sb[:, c0 : c0 + w], ostage[0:CQ, :],
                                 rcp_bc[:, 0:w])

    norm_tiles = {}

    def normalize_half(ci, out_ps, h):
        # Mid-chunk normalize, one 512-wide half at a time. Half 0 is
        # issued at the chunk's end; half 1 from inside the NEXT chunk's
        # k loop, a few units in: both halves are ready at the boundary,
        # so only priority (issue order) decides what the list scheduler
        # hoists into the boundary — deferring half 1 puts the next
        # chunk's first exps ahead of it in the ready-queue tie-breaks,
        # halving the boundary-resident ostage/recip work.
        c0, w = CHUNKS[ci]
        hw2 = w // 2
        hs = slice(h * hw2, (h + 1) * hw2)
        if h == 0:
            norm_tiles[ci] = (
                sbuf.tile([CQ + 1, 1024], F32, tag="ostage", bufs=1,
                          name=f"ostage_{ci}"),
                sbuf.tile([1, 1024], F32, tag="rcp", name=f"rcp_{ci}"),
            )
        ostage, rcp_sb = norm_tiles[ci]
        nc.scalar.copy(ostage[:, hs], out_ps[0 : CQ + 1, hs])
        nc.vector.reciprocal(rcp_sb[:, hs], ostage[CQ : CQ + 1, hs])
        nc.gpsimd.partition_broadcast(rcp_bc[:, hs], rcp_sb[:, hs])
        nc.gpsimd.tensor_mul(attn_sb[:, c0 + h * hw2 : c0 + (h + 1) * hw2],
                             ostage[0:CQ, hs], rcp_bc[:, hs])

    def phase4_unit(ps_pool, ci, oc, spread=True):
        # final projection + residual for one 128-row output chunk
        c0, w = CHUNKS[ci]
        if spread:
            # z halves borrow score slots (exp lookahead briefly 6 -> 4);
            # each half's add fires as soon as its matmul lands, DVE taking
            # one half and Pool the other
            y_sb = sbuf.tile([128, 1024], F32, tag="y", bufs=3,
                             name=f"y_sb_{ci}_{oc}")[:, 0:w]
            for g, (off, pw) in enumerate(pieces(w)):
                z_ps = ps_pool.tile([128, 512], F32, tag="s", bufs=S_BUFS,
                                    name=f"z_ps_{ci}_{oc}_{g}")[:, 0:pw]
                nc.tensor.matmul(
                    z_ps,
                    wl_sb[:, oc * 128 : (oc + 1) * 128],
                    attn_sb[:, c0 + off : c0 + off + pw],
                    start=True,
                    stop=True,
                )
                nc.vector.tensor_add(y_sb[:, off : off + pw], z_ps,
                               x_sb[:, oc, c0 + off : c0 + off + pw].bitcast(F32))
            nc.sync.dma_start(out=y_r[:, oc, slice(c0, c0 + w)], in_=y_sb)
            return
        # late-issued unit: residual via identity-matmul accumulate and a
        # ScalarE copy — its matmuls fill the last chunk's normalize-wait
        # PE gap, and it keeps the ~1.2us DVE add out of the tail window
        y_sb = sbuf.tile([128, 1024], F32, tag="y", bufs=3,
                         name=f"y_sb_{ci}_{oc}")[:, 0:w]
        for g, (off, pw) in enumerate(pieces(w)):
            zp = ps_pool.tile([128, 512], F32, tag="s", bufs=S_BUFS,
                              name=f"z_late_{ci}_{oc}_{g}")[:, 0:pw]
            nc.tensor.matmul(zp, ident_r, x_sb[:, oc, c0 + off : c0 + off + pw],
                             start=True, stop=False)
            nc.tensor.matmul(
                zp,
                wl_sb[:, oc * 128 : (oc + 1) * 128],
                attn_sb[:, c0 + off : c0 + off + pw],
                start=False,
                stop=True,
            )
            nc.scalar.copy(y_sb[:, off : off + pw], zp)
        nc.sync.dma_start(out=y_r[:, oc, slice(c0, c0 + w)], in_=y_sb)

    def phase4_last(ps_pool, ci, out_ps):
        # Last chunk: normalize from PSUM in two 256-wide halves (both
        # reciprocals on DVE up front, broadcasts on Pool, then the muls)
        # so the first projection matmuls start ~1us earlier than a
        # full-width chain; residual via identity-matmul accumulates
        # issued during the normalize latency; PSUM->SBUF bf16 copies
        # split 3 ScalarE / 3 DVE feeding three paired DMAs on SP (each
        # group one ScalarE + one DVE copy, so the last DMA issues as
        # early as either engine allows).
        c0, w = CHUNKS[ci]
        hw = w // 2
        zs = []
        for oc in range(KC):
            z_ps = ps_pool.tile([128, 512], F32, tag="s", bufs=S_BUFS,
                                name=f"z_ps_{ci}_{oc}")[:, 0:w]
            nc.tensor.matmul(
                z_ps,
                ident_r,
                x_sb[:, oc, c0 : c0 + w],
                start=True,
                stop=False,
            )
            zs.append(z_ps)
        # one rcp tile sliced per half: separate ring allocations would
        # serialize recip1 behind bcast0's read of the single-buf slot
        rcp2 = sbuf.tile([1, 1024], F32, tag="rcp", name=f"rcp_{ci}")
        for h in range(2):
            hs = slice(h * hw, (h + 1) * hw)
            nc.vector.reciprocal(rcp2[:, hs], out_ps[CQ : CQ + 1, hs])
            nc.gpsimd.partition_broadcast(rcp_bc[:, hs], rcp2[:, hs])
        for h in range(2):
            hs = slice(h * hw, (h + 1) * hw)
            nc.vector.tensor_mul(attn_sb[:, c0 + h * hw : c0 + (h + 1) * hw],
                                 out_ps[0:CQ, hs], rcp_bc[:, hs])
            for oc in range(KC):
                nc.tensor.matmul(
                    zs[oc][:, hs],
                    wl_sb[:, oc * 128 : (oc + 1) * 128],
                    attn_sb[:, c0 + h * hw : c0 + (h + 1) * hw],
                    start=False,
                    stop=(h == 1),
                )
        ysg = [sbuf.tile([128, 2, 512], BF16, tag=f"ylast{g}", bufs=1,
                         name=f"y_last_{g}") for g in range(3)]
        for g in range(3):
            nc.scalar.copy(ysg[g][:, 0, :], zs[2 * g])
            nc.vector.tensor_copy(ysg[g][:, 1, :], zs[2 * g + 1])
        # SP queue only: a ScalarE-issued DMA would block ScalarE's in-order
        # queue and delay the remaining copies
        for g in range(3):
            nc.sync.dma_start(out=y2_r[:, 2 * g : 2 * g + 2, :], in_=ysg[g])

    # ---- phase 1 + attention chunk 0 (512 wide), interleaved ------------
    # projections run in 512-column groups; as each group's K/V land, the
    # corresponding k-blocks of chunk 0 are scored/exp'd/accumulated.
    with (
        tc.tile_pool(name="ps_proj", bufs=1, space="PSUM") as ps_proj,
        tc.tile_pool(name="ps_aux", bufs=2, space="PSUM") as ps_aux,
    ):
        out0_ps = ps_proj.tile([128, 512], F32, tag="o0", name="out0_ps")
        # PE p-state warmup: the clock runs at half rate until 3us of
        # CONTINUOUS busy, and the first x piece only lands ~3.9us in.
        # Dummy identity transposes (no readers, recycled ps_aux ring)
        # keep the PE busy from ~0.3us so the ramp completes before the
        # first projection matmul — which then runs at the full 2.4GHz.
        for wu in range(CFG.get("warmup", 0)):
            wu_ps = ps_aux.tile([128, 128], F32, tag="sm", name=f"wu_{wu}")
            nc.tensor.transpose(wu_ps, ident, ident)
        pend_attnv = []  # attnV lag FIFO so PE never waits on exp in-order
        for gp in range(8):
            gs = slice(gp * 512, (gp + 1) * 512)
            tiles = [
                ps_proj.tile([CQ, 512], F32, tag=f"proj{t}", name=f"p_ps_{t}_{gp}")
                for t in range(3)
            ]
            for kc in range(KC):
                for t in range(3):
                    if gp == 0 and kc == 0 and CFG.get("split_start", True):
                        # x arrives in two 256-col halves; start on the first
                        for ho in (0, 256):
                            nc.tensor.matmul(
                                tiles[t][:, ho : ho + 256],
                                w_sb[:, 0, t * CQ : (t + 1) * CQ],
                                x_sb[:, 0, ho : ho + 256],
                                start=True,
                                stop=False,
                                skip_group_check=True,
                            )
                        continue
                    nc.tensor.matmul(
                        tiles[t],
                        w_sb[:, kc, t * CQ : (t + 1) * CQ],
                        x_sb[:, kc, gs],
                        start=(kc == 0),
                        stop=(kc == KC - 1),
                        skip_group_check=(gp == 0),
                    )
            for t, dst in ((0, q_sb), (1, k_sb), (2, v_sb)):
                if t == 1:
                    nc.vector.tensor_copy(dst[:, gs], tiles[t])
                else:
                    nc.scalar.copy(dst[:, gs], tiles[t])

            # chunk-0 attention for this group's 4 k-blocks
            for kb in range(4 * gp, 4 * gp + 4):
                s_ps = ps_proj.tile([128, 512], F32, tag="s0", bufs=2,
                                    name=f"s_ps_0_{kb}")
                scores_mms(s_ps, kb, 0, 512)
                if kb == 0:
                    # shift estimate from these 65k scores (statistically
                    # ample for a shift that merely has to land within
                    # ~+-80 of the true max)
                    nc.vector.reduce_max(m_row, s_ps, axis=AX.X)
                    mt_ps = ps_aux.tile([1, 128], F32, tag="sm")
                    nc.tensor.transpose(mt_ps, m_row, ident)
                    nc.vector.tensor_copy(mt_sb[:, 0:128], mt_ps)
                    nc.vector.reduce_max(small[0:1, 11:12], mt_sb[:, 0:128],
                                         axis=AX.X)
                    nc.gpsimd.partition_broadcast(gmax_bc, small[0:1, 11:12])
                    # neg_c = -(gmax + MARGIN)
                    nc.scalar.activation(neg_c, gmax_bc, AF.Copy,
                                         bias=-MARGIN, scale=-1.0)
                    # b_eff = SCH_A * neg_c + SCH_B (for the approx engines)
                    nc.vector.tensor_scalar(b_eff, neg_c, SCH_A, SCH_B,
                                            OP.mult, OP.add)
                et = exp_tile(0, kb, s_ps, 512, n_units=NKB)
                if len(pend_attnv) >= attnv_lag(0):
                    attnv_mm(out0_ps, *pend_attnv.pop(0))
                pend_attnv.append((et, kb, 0, 512))
            # V -> V^T transposes for this group's 4 l-blocks (the last
            # group's copies optionally on ScalarE so DVE is free for the
            # bridge exps that gate the pool swap)
            for lb in range(4 * gp, 4 * gp + 4):
                t_ps = ps_aux.tile([128, CQ], BF16, tag="sm", name=f"t_ps_{lb}")
                nc.tensor.transpose(
                    t_ps, v_sb[:, lb * 128 : (lb + 1) * 128], ident_bf[0:CQ, 0:CQ]
                )
                if gp == 7 and CFG.get("vt7_scalar"):
                    nc.scalar.copy(vt_sb[:, lb, 0:CQ], t_ps)
                else:
                    nc.vector.tensor_copy(vt_sb[:, lb, 0:CQ], t_ps)

        for pa in pend_attnv:
            attnv_mm(out0_ps, *pa)
        # chunk-0 normalize first: its ScalarE staging copy releases the
        # out0 banks so the pool-close barrier (gating all of chunk 1) isn't
        # stuck behind the bridge exps
        normalize(0, out0_ps)
        # bridge: score+exp chunk-1's k-block 0 in this pool's slots so
        # ScalarE never idles across the PSUM pool swap
        bridge_units = []
        for bu in range(2 * CFG.get("bridge_kb", 1)):
            kb, h = bu // 2, bu % 2
            sb_ps = ps_proj.tile([128, 512], F32, tag="s0", bufs=2,
                                 name=f"sb_ps_{bu}")
            nc.tensor.matmul(
                sb_ps, k_sb[:, kb * 128 : (kb + 1) * 128],
                q_sb[:, 512 + h * 512 : 512 + (h + 1) * 512],
                start=True, stop=True,
            )
            bet = sbuf.tile([128, 512], BF16, tag="et", bufs=ET_BUFS,
                            name=f"et_1_0_{bu}")
            # engine choice: the pool close (gating all of chunk 1) waits on
            # these exps' PSUM reads, racing the other engine's backlog
            if CFG.get("bridge_eng", "vv")[bu % len(CFG.get("bridge_eng", "vv"))] == 'v':
                nc.vector.tensor_scalar(bet.bitcast(U16), sb_ps, SCH_A, b_eff,
                                        OP.mult, OP.add)
            else:
                nc.scalar.activation(bet, sb_ps, AF.Exp, bias=neg_c, scale=1.0)
            bridge_units.append((bet, kb, h * 512, 512))

    # ---- attention chunks 1..4 ------------------------------------------
    with tc.tile_pool(name="ps_attn", bufs=1, space="PSUM") as ps_attn:
        prev_ps = [None]   # previous chunk's accumulator awaiting half-1
        for ci in range(1, len(CHUNKS)):
            c0, w = CHUNKS[ci]
            out_ps = ps_attn.tile(
                [128, 1024], F32, tag="obig", bufs=1, name=f"out_ps_{ci}"
            )
            # attnV trails exp by ATTNV_LAG 512-wide units; 4 s_ps slots let
            # up to 4 exps run concurrently across ScalarE/DVE/Pool
            pend = list(bridge_units) if ci == 1 else []
            nh = len(pieces(w))
            for kb in range(CFG.get("bridge_kb", 1) if ci == 1 else 0, NKB):
                for h, (off, pw) in enumerate(pieces(w)):
                    u = kb * nh + h
                    s_ps = ps_attn.tile(
                        [128, 512], F32, tag="s", bufs=S_BUFS,
                        name=f"s_ps_{ci}_{u}"
                    )[:, 0:pw]
                    nc.tensor.matmul(
                        s_ps,
                        k_sb[:, kb * 128 : (kb + 1) * 128],
                        q_sb[:, c0 + off : c0 + off + pw],
                        start=True,
                        stop=True,
                    )
                    et = exp_tile(ci, u, s_ps, pw, n_units=NKB * nh)
                    if len(pend) >= attnv_lag(ci):
                        attnv_mm(out_ps, *pend.pop(0))
                    pend.append((et, kb, off, pw))
                    if u == CFG.get("h1_at", 3) and ci >= 2 and \
                            CFG.get("split_norm", False):
                        normalize_half(ci - 1, prev_ps[0], 1)
                # spread the previous chunk's phase 4 through this chunk's
                # k loop, starting at kb=8 so the previous chunk's normalize
                # chain (which the z matmuls depend on) has finished — PE is
                # in-order, so an early-enqueued z matmul would stall scores
                last = ci == len(CHUNKS) - 1
                sp0, step = CFG.get("sp_last", (16, 3)) if last else CFG.get("sp_mid", (12, 3))
                # all spread units must fit inside this chunk's kb range — a
                # unit past kb=31 would silently drop an output block
                nsp = KC - 1 if (last and CFG.get("late6", True)) else KC
                assert sp0 + step * (nsp - 1) < NKB
                if (kb - sp0) % step == 0 and sp0 <= kb < sp0 + step * nsp:
                    phase4_unit(ps_attn, ci - 1, (kb - sp0) // step)
            for pe in pend:
                attnv_mm(out_ps, *pe)
            if ci == len(CHUNKS) - 1 and CFG.get("late6", True):
                # chunk-3's last output block, issued after the drain: its
                # matmuls fill the PE gap while the tail normalize runs
                phase4_unit(ps_attn, ci - 1, KC - 1, spread=False)
            if ci < len(CHUNKS) - 1:
                if CFG.get("split_norm", False):
                    normalize_half(ci, out_ps, 0)
                    prev_ps[0] = out_ps
                else:
                    normalize(ci, out_ps)

        # last chunk's normalize halves + phase 4
        phase4_last(ps_attn, len(CHUNKS) - 1, out_ps)


def build(loop_iters=1):
    nc = bacc.Bacc("TRN2", target_bir_lowering=False, debug=False, num_devices=8)
    x = nc.dram_tensor("x", [C, L], F32, kind="ExternalInput").ap()
    wqkv_t = nc.dram_tensor("wqkv_t", [C, 3 * CQ], F32, kind="ExternalInput").ap()
    wl_t = nc.dram_tensor("wl_t", [CQ, C], F32, kind="ExternalInput").ap()
    y = nc.dram_tensor("y", [C, L], F32, kind="ExternalOutput").ap()
    # last q-chunk's output in bf16: halves the tail's DMA wire time; the
    # 0.39% bf16 step is well under the error budget (host converts back)
    y2 = nc.dram_tensor("y2", [C, CHUNKS[-1][1]], mybir.dt.bfloat16,
                        kind="ExternalOutput").ap()

    with tile.TileContext(nc) as tc:
        with tc.tile_pool(name="sbuf", bufs=1) as sbuf:
            if loop_iters > 1:
                engines = (
                    mybir.EngineType.PE,
                    mybir.EngineType.Activation,
                    mybir.EngineType.DVE,
                    mybir.EngineType.Pool,
                    mybir.EngineType.SP,
                )
                with tc.For_i(0, loop_iters, hint_engines=engines):
                    body(nc, tc, sbuf, x, wqkv_t, wl_t, y, y2)
            else:
                body(nc, tc, sbuf, x, wqkv_t, wl_t, y, y2)

    nc.compile()
    return nc


_cached_nc = None


def kernel(x, Wq, Wk, Wv, Wlast, gamma):
    global _cached_nc
    x = np.ascontiguousarray(np.asarray(x, dtype=np.float32))
    B = x.shape[0]
    assert B == 8 and x.shape[1:] == (C, 64, 64)
    wqkv_t = np.ascontiguousarray(
        np.concatenate([Wq, Wk, Wv], axis=0).T.astype(np.float32)
    )
    wl_t = np.ascontiguousarray(
        (np.asarray(Wlast, np.float32) * np.float32(np.asarray(gamma)[0])).T
    )

    if _cached_nc is None:
        _cached_nc = build()
    nc = _cached_nc

    in_maps = [
        {
            "x": np.ascontiguousarray(x[b].reshape(C, L)),
            "wqkv_t": wqkv_t,
            "wl_t": wl_t,
        }
        for b in range(B)
    ]
    res = bass_utils.run_bass_kernel_spmd(nc, in_maps, core_ids=list(range(B)))
    lw = CHUNKS[-1][1]
    outs = []
    for b in range(B):
        yb = np.array(res.results[b]["y"]).reshape(C, L)
        yb[:, L - lw:] = res.results[b]["y2"].astype(np.float32)
        outs.append(yb.reshape(C, 64, 64))
    return np.stack(outs).astype(np.float32)



# revision 78
# speedup vs baseline: 1.0022x; 1.0010x over previous
"""Trainium2 Bass kernel for nn_AttentionLayer_47596827574368.

Reference computation (per batch sample b, B=8, C=768, H=W=64, L=4096, Cqk=Cv=96):
  Q = Wq @ X, K = Wk @ X, V = Wv @ X            (X = x[b] as [C, L])
  S = Q^T K   [L, L];  beta = softmax(S, axis=-1)
  O = beta @ V^T      [L, Cv]
  y = gamma * (Wlast @ O^T) + X                 [C, L]

Sharding: data-parallel over batch — one sample per NeuronCore (8 cores).

Device plan (per core):
  X streamed in (chunk, 512-col) pieces; Q/K c-major [96, 4096]; V^T as 32
  blocks [128(k), 97] (col 96 = ones -> softmax denominators ride along in
  the attnV matmul); scores computed transposed S^T[k, q] per 128-k block.
  Softmax uses a global-shift exp (C = est_max + 8 sampled from k-block 0;
  exact per-row max is unnecessary: softmax is shift-invariant and fp32 exp
  has huge dynamic-range headroom). Normalization is applied before the
  final projection; gamma is folded into Wlast on the host.

  The 16.7M-element exp of the score matrix is the ScalarE bottleneck
  (~1ns/element at 1.2GHz, vs the PE's 0.83ns/element of matmul alongside),
  so the PSUM score drain is split: ScalarE runs exact exp on ~2/3 of the
  512-wide units, VectorE runs a one-instruction Schraudolph approximation
  on the rest (uint16(A*s + B) bitcast as bf16, max rel err ~3.3%;
  fp32->uint16 conversion rounds and saturates negatives to 0, so
  deep-negative scores become +0.0). Numerator and denominator use the same
  approximate weights, so softmax stays a proper weighted average; measured
  end-to-end error is 8.6e-3 (budget 2e-2). GPSIMD/Pool cannot read PSUM
  on TRN2, so it only handles the SBUF-side reciprocal broadcasts.

  q columns are processed in chunks [512, 1024, 1024, 1024, 512]: the first
  512 chunk is interleaved with the projection phase, the last 512 chunk
  keeps the final drain short. Scores/exp/attnV run at 512-wide unit
  granularity through 6 single-bank PSUM score slots (up to 6 exps in
  flight across both drain engines); the attnV accumulation trails exp by a
  deep unit lag to ride out exp-latency spikes. Chunk normalization runs
  reciprocal on DVE, broadcast + multiply on Pool off a ScalarE-staged
  SBUF copy. (A Pool-side ones/denom divide simmed 0.2us faster but the
  gpsimd divide op fails the real device compile — TimelineSim does not
  validate op support, so HW-verify any new op type.)
  Each chunk's final projection + residual is spread through the
  next chunk's k loop (residual adds on DVE). The last chunk's tail is the
  critical serial path: it normalizes straight from PSUM in two 256-wide
  halves — both reciprocals issued back-to-back on DVE into disjoint
  slices of ONE rcp tile (separate single-buf ring allocations would
  serialize recip1 behind broadcast0's read), broadcasts on Pool, muls on
  DVE — so the Wlast projection matmuls start ~1.5us earlier than a
  full-width chain; the residual arrives via identity-matmul accumulates
  issued during the normalize latency; the drain is 3 ScalarE + 3 DVE
  bf16 copies feeding three paired DMAs on SP (each group one ScalarE +
  one DVE copy; bf16 halves the tail wire, and the 0.39% step is far
  under the error budget — the host converts back and stitches). Matmuls
  run in float32r (full PE rate). PSUM: proj phase 3+1+2 banks (+2 aux);
  main phase 6 score slots + 2 accumulator banks.

  Note for future tuning: the Tile framework list-schedules per engine by
  readiness (program order is only a tie-break), so "issue X later" code
  motion does NOT delay X — boundary ops like the ostage copy hoist into
  any engine idle slot as soon as their inputs are ready. A 1024-wide
  paired-exp variant (one exp per two 512 units; see kernel_pair.py) cuts
  ~17us of drain-engine busy time but loses ~7us to coarser score-slot
  recycling at chunk boundaries — net worse while the PE, not the drain
  engines, is the bottleneck (~91% busy).

  The last phase4 output block of chunk 3 is issued late (after the last
  chunk's attnV drain, identity-residual + ScalarE-copy form): its four
  matmuls fill the PE gap while the tail normalize chain runs, and its
  ~1.2us DVE add stays out of the tail window. The first of the last
  chunk's four trailing exps runs on DVE (tailpat 'vaaa') so the z-ring
  slots recycle without serializing behind ScalarE's backlog.

  The shift estimate samples k-block 0's first 256 q columns (32k scores
  — statistically ample, and the reduce_max sits on the chunk-0 exp
  critical path; halving it from 512 was worth ~170ns).

  Timeline-sim: 168921 ns/core (prior session: 171303, original baseline:
  198420); measured HW rel err 8.5e-3 (budget 2e-2). The exp engine
  schedules (vset {1,4}, midtail 'aava', tailpat 'vaaa', c0tail 'avav')
  came from exhaustive joint sweeps — re-sweep them all after ANY
  structural change; their optima shift and stale settings cost ~500ns.
"""

import numpy as np

import concourse.bass as bass
import concourse.tile as tile
import concourse.mybir as mybir
from concourse import bacc
from concourse import bass_utils
from concourse.masks import make_identity

F32 = mybir.dt.float32
F32R = mybir.dt.float32r
BF16 = mybir.dt.bfloat16
U16 = mybir.dt.uint16
AF = mybir.ActivationFunctionType
AX = mybir.AxisListType
OP = mybir.AluOpType

C = 768          # input/output channels
CQ = 96          # qk/v channels
L = 4096         # H*W
KC = C // 128    # 6 contraction chunks
NKB = L // 128   # 32 k blocks
MARGIN = 8.0     # exp shift safety margin

# Schraudolph bf16-bits exp: exp(x) ~ bitcast_bf16(uint16(SCH_A*x + SCH_B)).
# Tuned on-device: fp32->uint16 converts round-to-nearest with saturation.
SCH_A = float(np.float32(128.0 / np.log(2.0)))
SCH_B = 16250.5

CHUNKS = [(0, 512), (512, 1024), (1536, 1024), (2560, 1024), (3584, 512)]
ET_BUFS = 12
S_BUFS = 6      # 512-wide score slots: exp latency tolerance / parallelism

# Schedule knobs (sweepable via sim_trace/sweep scripts):
CFG = {
    "lag": (8, 12, 4),       # attnV unit lag for (chunk0, mid, last)
    "vset": (1, 4),          # u%8 residues DVE takes in mid chunks
    "tailpat": "vaaa",       # engines of the last chunk's last 4 exps
    "bridge_eng": "av",      # bridge exp engines (pool-close race)
    "split_start": False,    # split the first x piece into 256-col halves
    "psum_recip": False,     # recip from PSUM denom row (worse in sim)
    "c0tail": "avav",        # engines of chunk 0's last 4 exps
    "midtail": "aava",       # engines of mid chunks' last 4 exps
    "est_w": 256,            # shift-estimate sample width (reduce_max cols)
}


def attnv_lag(ci):
    """attnV trails exp by this many 512-wide units. Chunk 0's drain gates
    the PSUM pool swap and the last chunk's drain gates the tail, so they
    may use shallower lags than the latency-tolerant mid chunks."""
    l0, lm, ll = CFG["lag"]
    if ci == 0:
        return l0
    if ci == len(CHUNKS) - 1:
        return ll
    return lm


def exp_engine(ci, u, n_units):
    """Which engine computes exp for 512-wide unit u (= kb*halves + h) of
    chunk ci. 'a' = ScalarE exact exp; 'v' = DVE Schraudolph approx (~1/3
    of units, validated end-to-end at ~7e-3 rel err).

    GPSIMD/Pool cannot read PSUM on TRN2 (BIR verifier rejects it), so only
    ScalarE and DVE can drain the score tiles. A chunk's first 8 units stay
    on ScalarE (DVE's queue holds the previous chunk's recip/mul then); two
    of the last 4 go to DVE so the trailing exps — which gate the next
    chunk's score slots — don't sit behind ScalarE's backlog."""
    if u >= n_units - 4:
        # trailing units gate the next chunk's score slots (or the tail's
        # z-ring); split them across both engines so neither serializes
        if ci == len(CHUNKS) - 1:
            return CFG.get("tailpat", "aaaa")[u % 4]
        if ci == 0:
            return CFG.get("c0tail", "vava")[u % 4]
        return CFG.get("midtail", "vava")[u % 4]
    if ci == 0:
        # proj phase: mostly ScalarE (it has slack at 512-wide tiles), with
        # a small DVE share so chunk-0's score slots never back up
        m, r, lo = CFG.get("c0v", (4, 1, 4))
        return 'v' if (u % m == r and u >= lo) else 'a'
    if u < 8:
        # alternate from the start: DVE only pays the previous chunk's
        # reciprocal now, so it can take every other early unit
        return CFG.get("head", "vavavava")[u]
    vs = CFG.get(f"vset{ci}") or CFG.get("vset", (2, 5, 7))
    if u % 8 in vs:
        return 'v'
    return 'a'


def pieces(w):
    # split a chunk width into matmul-sized pieces (<=512, >=256 so f32r
    # stays at full rate and no PSUM bank is crossed)
    out = []
    off = 0
    while w - off > 512:
        out.append((off, 512))
        off += 512
    out.append((off, w - off))
    return out


def body(nc, tc, sbuf, x, wqkv_t, wl_t, y, y2):
    # ---- persistent sbuf tiles -----------------------------------------
    # weights first (per-kc pieces so the first proj matmul starts early)
    w_sb = sbuf.tile([128, KC, 3 * CQ], F32R, tag="w")
    w_r = wqkv_t.rearrange("(ko ki) m -> ki ko m", ki=128).bitcast(F32R)
    # DMA issue costs ~650ns of sequencer time apiece, so startup spreads
    # issues across queues: weights on Pool's SWDGE, x on SP — the first
    # w slice (just Wq kc=0) and the first x piece reach the (shared,
    # serial) HWDGE back-to-back instead of ~1.3us apart; the identity
    # setup must come AFTER these issues (its Pool memsets would delay the
    # w SWDGE descriptor generation by ~1.5us -> +0.9us end-to-end)
    nc.gpsimd.dma_start(out=w_sb[:, 0, 0:CQ], in_=w_r[:, 0, 0:CQ])

    x_sb = sbuf.tile([128, KC, L], F32R, tag="x")
    x_r = x.rearrange("(ko ki) l -> ki ko l", ki=128).bitcast(F32R)
    # first piece split in half: the first projection matmuls run on
    # [*, 0:256] and start ~0.35us earlier (half the first-piece wire)
    if CFG.get("split_start", True):
        nc.sync.dma_start(out=x_sb[:, 0, 0:256], in_=x_r[:, 0, 0:256])
        nc.scalar.dma_start(out=w_sb[:, 0, CQ:], in_=w_r[:, 0, CQ:])
        nc.sync.dma_start(out=x_sb[:, 0, 256:512], in_=x_r[:, 0, 256:512])
    elif CFG.get("x0_swdge"):
        # first piece through the software DGE on Pool's queue: ~25ns seq
        # + ~1us fixed vs the HWDGE path's ~0.6+1.8us — lands ~0.8us sooner
        nc.gpsimd.dma_start(out=x_sb[:, 0, 0:512], in_=x_r[:, 0, 0:512])
        nc.scalar.dma_start(out=w_sb[:, 0, CQ:], in_=w_r[:, 0, CQ:])
    else:
        nc.sync.dma_start(out=x_sb[:, 0, 0:512], in_=x_r[:, 0, 0:512])
        nc.scalar.dma_start(out=w_sb[:, 0, CQ:], in_=w_r[:, 0, CQ:])
    # interleave the remaining weight pieces between group-0 x pieces in
    # demand order — weights queued up-front would push the x stream (the
    # projection pacer) back by ~3us
    if CFG.get("merge_w", False):
        # one merged DMA for w2..w5 frees three serial ~625ns HWDGE slots,
        # pulling every group-1/2 x piece earlier (they pace the PE around
        # 10-13us); w1 stays separate so kc=1's matmul isn't gated on the
        # whole merged wire
        nc.sync.dma_start(out=x_sb[:, 1, 0:512], in_=x_r[:, 1, 0:512])
        nc.scalar.dma_start(out=w_sb[:, 1, :], in_=w_r[:, 1, :])
        nc.sync.dma_start(out=x_sb[:, 2, 0:512], in_=x_r[:, 2, 0:512])
        nc.scalar.dma_start(out=w_sb[:, 2:, :], in_=w_r[:, 2:, :])
        for kc in range(3, KC):
            nc.sync.dma_start(out=x_sb[:, kc, 0:512], in_=x_r[:, kc, 0:512])
    else:
        for kc in range(1, KC):
            nc.sync.dma_start(out=x_sb[:, kc, 0:512], in_=x_r[:, kc, 0:512])
            nc.scalar.dma_start(out=w_sb[:, kc, :], in_=w_r[:, kc, :])
    wl_sb = sbuf.tile([CQ, C], F32R, tag="wl")
    # remaining groups in consumption order (wl after group 1 — first
    # needed ~60us in, by phase4 of chunk 0)
    for gp in range(1, 8):
        gs = slice(gp * 512, (gp + 1) * 512)
        for kc in range(KC):
            nc.sync.dma_start(out=x_sb[:, kc, gs], in_=x_r[:, kc, gs])
        if gp == CFG.get("wl_after", 1):
            nc.scalar.dma_start(out=wl_sb, in_=wl_t.bitcast(F32R))

    ident = sbuf.tile([128, 128], F32, tag="ident")
    make_identity(nc, ident)
    ident_bf = sbuf.tile([128, 128], BF16, tag="identbf")
    make_identity(nc, ident_bf)
    # f32r copy of the identity (the residual-add matmul needs an f32r
    # producer; a plain bitcast of the F32 tile fails BIR verification)
    ident_r = sbuf.tile([128, 128], F32R, tag="identr")
    nc.scalar.copy(ident_r, ident)

    q_sb = sbuf.tile([CQ, L], F32R, tag="q")
    k_sb = sbuf.tile([CQ, L], F32R, tag="k")
    v_sb = sbuf.tile([CQ, L], BF16, tag="vbig")
    vt_sb = sbuf.tile([128, NKB, CQ + 1], BF16, tag="vt")
    # ones column (f32r producer required: memset can't write f32r)
    nc.scalar.activation(
        out=vt_sb[:, :, CQ : CQ + 1].rearrange("p a b -> p (a b)"),
        in_=ident[:, 0:NKB],
        func=AF.Copy,
        bias=1.0,
        scale=0.0,
    )

    small = sbuf.tile([128, 16], F32, tag="small")
    m_row = small[:, 8:9]
    neg_c = small[:, 9:10]
    gmax_bc = small[:, 10:11]
    b_eff = small[:, 12:13]       # SCH_A * neg_c + SCH_B  (per partition)
    mt_sb = sbuf.tile([1, 128], F32, tag="rcp")

    attn_sb = sbuf.tile([CQ, L], F32R, tag="vbig", name="attn_sb")
    rcp_bc = sbuf.tile([CQ, 1024], F32, tag="rbc")
    y_r = y.rearrange("(ko ki) l -> ki ko l", ki=128)
    y2_r = y2.rearrange("(ko ki) l -> ki ko l", ki=128)

    def scores_mms(s_ps, kb, c0, w):
        for off, pw in pieces(w):
            nc.tensor.matmul(
                s_ps[:, off : off + pw],
                k_sb[:, kb * 128 : (kb + 1) * 128],
                q_sb[:, c0 + off : c0 + off + pw],
                start=True,
                stop=True,
            )

    def attnv_mm(out_ps, et, kb, off, pw):
        nc.tensor.matmul(
            out_ps[0 : CQ + 1, off : off + pw],
            vt_sb[:, kb, :],
            et[:, 0:pw],
            start=(kb == 0),
            stop=(kb == NKB - 1),
        )

    def exp_tile(ci, u, s_ps, pw, n_units=NKB * 2):
        """exp(s - C) into a bf16 et tile, on the engine exp_engine says."""
        et = sbuf.tile([128, 512], BF16, tag="et", bufs=ET_BUFS,
                       name=f"et_{ci}_{u}")[:, 0:pw]
        eng = exp_engine(ci, u, n_units)
        if eng == 'a':
            nc.scalar.activation(et, s_ps, AF.Exp, bias=neg_c, scale=1.0)
        else:
            nc.vector.tensor_scalar(et.bitcast(U16), s_ps, SCH_A, b_eff,
                                    OP.mult, OP.add)
        return et

    def normalize(ci, out_ps):
        #   attn[:, c0:c0+w] = out_ps[0:96] * (1 / out_ps[96])
        # Mid chunks read straight from PSUM (no staging copy): the
        # accumulator's banks are only needed again two chunks later, and
        # skipping the copy cuts the serial recip -> broadcast -> mul latency
        # at each chunk boundary. Chunk 0 lives in the ps_proj pool whose
        # close barrier gates all of chunk 1, so it stages through SBUF with
        # one fast ScalarE copy to release its banks immediately.
        c0, w = CHUNKS[ci]
        if ci == len(CHUNKS) - 1:
            # tail: DVE is otherwise idle; straight from PSUM is the
            # shortest chain before phase4
            rcp_sb = sbuf.tile([1, 1024], F32, tag="rcp",
                               name=f"rcp_{ci}")[:, 0:w]
            nc.vector.reciprocal(rcp_sb, out_ps[CQ : CQ + 1, 0:w])
            nc.gpsimd.partition_broadcast(rcp_bc[:, 0:w], rcp_sb)
            nc.vector.tensor_mul(attn_sb[:, c0 : c0 + w], out_ps[0:CQ, 0:w],
                                 rcp_bc[:, 0:w])
        else:
            # stage through SBUF (ScalarE) and multiply on Pool: DVE only
            # pays the reciprocal, so its exp share starts ~2us earlier at
            # each chunk boundary — where ScalarE alone can't keep the
            # score slots draining at the PE's pace. Also frees the
            # accumulator's banks early (obig has a single slot).
            # the denominator row rides in the staged copy, so the
            # reciprocal reads SBUF — for chunk 0 that takes it off the
            # ps_proj pool-close path (which gates all of chunk 1)
            ostage = sbuf.tile([CQ + 1, 1024], F32, tag="ostage", bufs=1,
                               name=f"ostage_{ci}")[:, 0:w]
            nc.scalar.copy(ostage, out_ps[0 : CQ + 1, 0:w])
            rcp_sb = sbuf.tile([1, 1024], F32, tag="rcp",
                               name=f"rcp_{ci}")[:, 0:w]
            nc.vector.reciprocal(rcp_sb, ostage[CQ : CQ + 1, :])
            nc.gpsimd.partition_broadcast(rcp_bc[:, 0:w], rcp_sb)
            nc.gpsimd.tensor_mul(attn_sb[:, c0 : c0 + w], ostage[0:CQ, :],
                                 rcp_bc[:, 0:w])

    norm_tiles = {}

    def normalize_half(ci, out_ps, h):
        # Mid-chunk normalize, one 512-wide half at a time. Half 0 is
        # issued at the chunk's end; half 1 from inside the NEXT chunk's
        # k loop, a few units in: both halves are ready at the boundary,
        # so only priority (issue order) decides what the list scheduler
        # hoists into the boundary — deferring half 1 puts the next
        # chunk's first exps ahead of it in the ready-queue tie-breaks,
        # halving the boundary-resident ostage/recip work.
        c0, w = CHUNKS[ci]
        hw2 = w // 2
        hs = slice(h * hw2, (h + 1) * hw2)
        if h == 0:
            norm_tiles[ci] = (
                sbuf.tile([CQ + 1, 1024], F32, tag="ostage", bufs=1,
                          name=f"ostage_{ci}"),
                sbuf.tile([1, 1024], F32, tag="rcp", name=f"rcp_{ci}"),
            )
        ostage, rcp_sb = norm_tiles[ci]
        nc.scalar.copy(ostage[:, hs], out_ps[0 : CQ + 1, hs])
        nc.vector.reciprocal(rcp_sb[:, hs], ostage[CQ : CQ + 1, hs])
        nc.gpsimd.partition_broadcast(rcp_bc[:, hs], rcp_sb[:, hs])
        nc.gpsimd.tensor_mul(attn_sb[:, c0 + h * hw2 : c0 + (h + 1) * hw2],
                             ostage[0:CQ, hs], rcp_bc[:, hs])

    def phase4_unit(ps_pool, ci, oc, spread=True):
        # final projection + residual for one 128-row output chunk
        c0, w = CHUNKS[ci]
        if spread:
            # z halves borrow score slots (exp lookahead briefly 6 -> 4);
            # each half's add fires as soon as its matmul lands, DVE taking
            # one half and Pool the other
            y_sb = sbuf.tile([128, 1024], F32, tag="y", bufs=3,
                             name=f"y_sb_{ci}_{oc}")[:, 0:w]
            for g, (off, pw) in enumerate(pieces(w)):
                z_ps = ps_pool.tile([128, 512], F32, tag="s", bufs=S_BUFS,
                                    name=f"z_ps_{ci}_{oc}_{g}")[:, 0:pw]
                nc.tensor.matmul(
                    z_ps,
                    wl_sb[:, oc * 128 : (oc + 1) * 128],
                    attn_sb[:, c0 + off : c0 + off + pw],
                    start=True,
                    stop=True,
                )
                nc.vector.tensor_add(y_sb[:, off : off + pw], z_ps,
                               x_sb[:, oc, c0 + off : c0 + off + pw].bitcast(F32))
            nc.sync.dma_start(out=y_r[:, oc, slice(c0, c0 + w)], in_=y_sb)
            return
        # late-issued unit: residual via identity-matmul accumulate and a
        # ScalarE copy — its matmuls fill the last chunk's normalize-wait
        # PE gap, and it keeps the ~1.2us DVE add out of the tail window
        y_sb = sbuf.tile([128, 1024], F32, tag="y", bufs=3,
                         name=f"y_sb_{ci}_{oc}")[:, 0:w]
        for g, (off, pw) in enumerate(pieces(w)):
            zp = ps_pool.tile([128, 512], F32, tag="s", bufs=S_BUFS,
                              name=f"z_late_{ci}_{oc}_{g}")[:, 0:pw]
            nc.tensor.matmul(zp, ident_r, x_sb[:, oc, c0 + off : c0 + off + pw],
                             start=True, stop=False)
            nc.tensor.matmul(
                zp,
                wl_sb[:, oc * 128 : (oc + 1) * 128],
                attn_sb[:, c0 + off : c0 + off + pw],
                start=False,
                stop=True,
            )
            nc.scalar.copy(y_sb[:, off : off + pw], zp)
        nc.sync.dma_start(out=y_r[:, oc, slice(c0, c0 + w)], in_=y_sb)

    def phase4_last(ps_pool, ci, out_ps):
        # Last chunk: normalize from PSUM in two 256-wide halves (both
        # reciprocals on DVE up front, broadcasts on Pool, then the muls)
        # so the first projection matmuls start ~1us earlier than a
        # full-width chain; residual via identity-matmul accumulates
        # issued during the normalize latency; PSUM->SBUF bf16 copies
        # split 3 ScalarE / 3 DVE feeding three paired DMAs on SP (each
        # group one ScalarE + one DVE copy, so the last DMA issues as
        # early as either engine allows).
        c0, w = CHUNKS[ci]
        hw = w // 2
        zs = []
        for oc in range(KC):
            z_ps = ps_pool.tile([128, 512], F32, tag="s", bufs=S_BUFS,
                                name=f"z_ps_{ci}_{oc}")[:, 0:w]
            nc.tensor.matmul(
                z_ps,
                ident_r,
                x_sb[:, oc, c0 : c0 + w],
                start=True,
                stop=False,
            )
            zs.append(z_ps)
        # one rcp tile sliced per half: separate ring allocations would
        # serialize recip1 behind bcast0's read of the single-buf slot
        rcp2 = sbuf.tile([1, 1024], F32, tag="rcp", name=f"rcp_{ci}")
        for h in range(2):
            hs = slice(h * hw, (h + 1) * hw)
            nc.vector.reciprocal(rcp2[:, hs], out_ps[CQ : CQ + 1, hs])
            nc.gpsimd.partition_broadcast(rcp_bc[:, hs], rcp2[:, hs])
        for h in range(2):
            hs = slice(h * hw, (h + 1) * hw)
            nc.vector.tensor_mul(attn_sb[:, c0 + h * hw : c0 + (h + 1) * hw],
                                 out_ps[0:CQ, hs], rcp_bc[:, hs])
            for oc in range(KC):
                nc.tensor.matmul(
                    zs[oc][:, hs],
                    wl_sb[:, oc * 128 : (oc + 1) * 128],
                    attn_sb[:, c0 + h * hw : c0 + (h + 1) * hw],
                    start=False,
                    stop=(h == 1),
                )
        ysg = [sbuf.tile([128, 2, 512], BF16, tag=f"ylast{g}", bufs=1,
                         name=f"y_last_{g}") for g in range(3)]
        for g in range(3):
            nc.scalar.copy(ysg[g][:, 0, :], zs[2 * g])
            nc.vector.tensor_copy(ysg[g][:, 1, :], zs[2 * g + 1])
        # SP queue only: a ScalarE-issued DMA would block ScalarE's in-order
        # queue and delay the remaining copies
        for g in range(3):
            nc.sync.dma_start(out=y2_r[:, 2 * g : 2 * g + 2, :], in_=ysg[g])

    # ---- phase 1 + attention chunk 0 (512 wide), interleaved ------------
    # projections run in 512-column groups; as each group's K/V land, the
    # corresponding k-blocks of chunk 0 are scored/exp'd/accumulated.
    with (
        tc.tile_pool(name="ps_proj", bufs=1, space="PSUM") as ps_proj,
        tc.tile_pool(name="ps_aux", bufs=2, space="PSUM") as ps_aux,
    ):
        out0_ps = ps_proj.tile([128, 512], F32, tag="o0", name="out0_ps")
        # PE p-state warmup: the clock runs at half rate until 3us of
        # CONTINUOUS busy, and the first x piece only lands ~3.9us in.
        # Dummy identity transposes (no readers, recycled ps_aux ring)
        # keep the PE busy from ~0.3us so the ramp completes before the
        # first projection matmul — which then runs at the full 2.4GHz.
        for wu in range(CFG.get("warmup", 0)):
            wu_ps = ps_aux.tile([128, 128], F32, tag="sm", name=f"wu_{wu}")
            nc.tensor.transpose(wu_ps, ident, ident)
        pend_attnv = []  # attnV lag FIFO so PE never waits on exp in-order
        for gp in range(8):
            gs = slice(gp * 512, (gp + 1) * 512)
            tiles = [
                ps_proj.tile([CQ, 512], F32, tag=f"proj{t}", name=f"p_ps_{t}_{gp}")
                for t in range(3)
            ]
            for kc in range(KC):
                for t in range(3):
                    if gp == 0 and kc == 0 and CFG.get("split_start", True):
                        # x arrives in two 256-col halves; start on the first
                        for ho in (0, 256):
                            nc.tensor.matmul(
                                tiles[t][:, ho : ho + 256],
                                w_sb[:, 0, t * CQ : (t + 1) * CQ],
                                x_sb[:, 0, ho : ho + 256],
                                start=True,
                                stop=False,
                                skip_group_check=True,
                            )
                        continue
                    nc.tensor.matmul(
                        tiles[t],
                        w_sb[:, kc, t * CQ : (t + 1) * CQ],
                        x_sb[:, kc, gs],
                        start=(kc == 0),
                        stop=(kc == KC - 1),
                        skip_group_check=(gp == 0),
                    )
            for t, dst in ((0, q_sb), (1, k_sb), (2, v_sb)):
                if t == 1:
                    nc.vector.tensor_copy(dst[:, gs], tiles[t])
                else:
                    nc.scalar.copy(dst[:, gs], tiles[t])

            # chunk-0 attention for this group's 4 k-blocks
            for kb in range(4 * gp, 4 * gp + 4):
                s_ps = ps_proj.tile([128, 512], F32, tag="s0", bufs=2,
                                    name=f"s_ps_0_{kb}")
                scores_mms(s_ps, kb, 0, 512)
                if kb == 0:
                    # shift estimate from these 65k scores (statistically
                    # ample for a shift that merely has to land within
                    # ~+-80 of the true max)
                    nc.vector.reduce_max(m_row, s_ps[:, 0:CFG.get('est_w', 512)], axis=AX.X)
                    mt_ps = ps_aux.tile([1, 128], F32, tag="sm")
                    nc.tensor.transpose(mt_ps, m_row, ident)
                    nc.vector.tensor_copy(mt_sb[:, 0:128], mt_ps)
                    nc.vector.reduce_max(small[0:1, 11:12], mt_sb[:, 0:128],
                                         axis=AX.X)
                    nc.gpsimd.partition_broadcast(gmax_bc, small[0:1, 11:12])
                    # neg_c = -(gmax + MARGIN)
                    nc.scalar.activation(neg_c, gmax_bc, AF.Copy,
                                         bias=-MARGIN, scale=-1.0)
                    # b_eff = SCH_A * neg_c + SCH_B (for the approx engines)
                    nc.vector.tensor_scalar(b_eff, neg_c, SCH_A, SCH_B,
                                            OP.mult, OP.add)
                et = exp_tile(0, kb, s_ps, 512, n_units=NKB)
                if len(pend_attnv) >= attnv_lag(0):
                    attnv_mm(out0_ps, *pend_attnv.pop(0))
                pend_attnv.append((et, kb, 0, 512))
            # V -> V^T transposes for this group's 4 l-blocks (the last
            # group's copies optionally on ScalarE so DVE is free for the
            # bridge exps that gate the pool swap)
            for lb in range(4 * gp, 4 * gp + 4):
                t_ps = ps_aux.tile([128, CQ], BF16, tag="sm", name=f"t_ps_{lb}")
                nc.tensor.transpose(
                    t_ps, v_sb[:, lb * 128 : (lb + 1) * 128], ident_bf[0:CQ, 0:CQ]
                )
                if gp == 7 and CFG.get("vt7_scalar"):
                    nc.scalar.copy(vt_sb[:, lb, 0:CQ], t_ps)
                else:
                    nc.vector.tensor_copy(vt_sb[:, lb, 0:CQ], t_ps)

        for pa in pend_attnv:
            attnv_mm(out0_ps, *pa)
        # chunk-0 normalize first: its ScalarE staging copy releases the
        # out0 banks so the pool-close barrier (gating all of chunk 1) isn't
        # stuck behind the bridge exps
        normalize(0, out0_ps)
        # bridge: score+exp chunk-1's k-block 0 in this pool's slots so
        # ScalarE never idles across the PSUM pool swap
        bridge_units = []
        for bu in range(2 * CFG.get("bridge_kb", 1)):
            kb, h = bu // 2, bu % 2
            sb_ps = ps_proj.tile([128, 512], F32, tag="s0", bufs=2,
                                 name=f"sb_ps_{bu}")
            nc.tensor.matmul(
                sb_ps, k_sb[:, kb * 128 : (kb + 1) * 128],
                q_sb[:, 512 + h * 512 : 512 + (h + 1) * 512],
                start=True, stop=True,
            )
            bet = sbuf.tile([128, 512], BF16, tag="et", bufs=ET_BUFS,
                            name=f"et_1_0_{bu}")
            # engine choice: the pool close (gating all of chunk 1) waits on
            # these exps' PSUM reads, racing the other engine's backlog
            if CFG.get("bridge_eng", "vv")[bu % len(CFG.get("bridge_eng", "vv"))] == 'v':
                nc.vector.tensor_scalar(bet.bitcast(U16), sb_ps, SCH_A, b_eff,
                                        OP.mult, OP.add)
            else:
                nc.scalar.activation(bet, sb_ps, AF.Exp, bias=neg_c, scale=1.0)
            bridge_units.append((bet, kb, h * 512, 512))

    # ---- attention chunks 1..4 ------------------------------------------
    with tc.tile_pool(name="ps_attn", bufs=1, space="PSUM") as ps_attn:
        prev_ps = [None]   # previous chunk's accumulator awaiting half-1
        for ci in range(1, len(CHUNKS)):
            c0, w = CHUNKS[ci]
            out_ps = ps_attn.tile(
                [128, 1024], F32, tag="obig", bufs=1, name=f"out_ps_{ci}"
            )
            # attnV trails exp by ATTNV_LAG 512-wide units; 4 s_ps slots let
            # up to 4 exps run concurrently across ScalarE/DVE/Pool
            pend = list(bridge_units) if ci == 1 else []
            nh = len(pieces(w))
            for kb in range(CFG.get("bridge_kb", 1) if ci == 1 else 0, NKB):
                for h, (off, pw) in enumerate(pieces(w)):
                    u = kb * nh + h
                    s_ps = ps_attn.tile(
                        [128, 512], F32, tag="s", bufs=S_BUFS,
                        name=f"s_ps_{ci}_{u}"
                    )[:, 0:pw]
                    nc.tensor.matmul(
                        s_ps,
                        k_sb[:, kb * 128 : (kb + 1) * 128],
                        q_sb[:, c0 + off : c0 + off + pw],
                        start=True,
                        stop=True,
                    )
                    et = exp_tile(ci, u, s_ps, pw, n_units=NKB * nh)
                    if len(pend) >= attnv_lag(ci):
                        attnv_mm(out_ps, *pend.pop(0))
                    pend.append((et, kb, off, pw))
                    if u == CFG.get("h1_at", 3) and ci >= 2 and \
                            CFG.get("split_norm", False):
                        normalize_half(ci - 1, prev_ps[0], 1)
                # spread the previous chunk's phase 4 through this chunk's
                # k loop, starting at kb=8 so the previous chunk's normalize
                # chain (which the z matmuls depend on) has finished — PE is
                # in-order, so an early-enqueued z matmul would stall scores
                last = ci == len(CHUNKS) - 1
                sp0, step = CFG.get("sp_last", (16, 3)) if last else CFG.get("sp_mid", (12, 3))
                # all spread units must fit inside this chunk's kb range — a
                # unit past kb=31 would silently drop an output block
                nsp = KC - 1 if (last and CFG.get("late6", True)) else KC
                assert sp0 + step * (nsp - 1) < NKB
                if (kb - sp0) % step == 0 and sp0 <= kb < sp0 + step * nsp:
                    phase4_unit(ps_attn, ci - 1, (kb - sp0) // step)
            for pe in pend:
                attnv_mm(out_ps, *pe)
            if ci == len(CHUNKS) - 1 and CFG.get("late6", True):
                # chunk-3's last output block, issued after the drain: its
                # matmuls fill the PE gap while the tail normalize runs
                phase4_unit(ps_attn, ci - 1, KC - 1, spread=False)
            if ci < len(CHUNKS) - 1:
                if CFG.get("split_norm", False):
                    normalize_half(ci, out_ps, 0)
                    prev_ps[0] = out_ps
                else:
                    normalize(ci, out_ps)

        # last chunk's normalize halves + phase 4
        phase4_last(ps_attn, len(CHUNKS) - 1, out_ps)


def build(loop_iters=1):
    nc = bacc.Bacc("TRN2", target_bir_lowering=False, debug=False, num_devices=8)
    x = nc.dram_tensor("x", [C, L], F32, kind="ExternalInput").ap()
    wqkv_t = nc.dram_tensor("wqkv_t", [C, 3 * CQ], F32, kind="ExternalInput").ap()
    wl_t = nc.dram_tensor("wl_t", [CQ, C], F32, kind="ExternalInput").ap()
    y = nc.dram_tensor("y", [C, L], F32, kind="ExternalOutput").ap()
    # last q-chunk's output in bf16: halves the tail's DMA wire time; the
    # 0.39% bf16 step is well under the error budget (host converts back)
    y2 = nc.dram_tensor("y2", [C, CHUNKS[-1][1]], mybir.dt.bfloat16,
                        kind="ExternalOutput").ap()

    with tile.TileContext(nc) as tc:
        with tc.tile_pool(name="sbuf", bufs=1) as sbuf:
            if loop_iters > 1:
                engines = (
                    mybir.EngineType.PE,
                    mybir.EngineType.Activation,
                    mybir.EngineType.DVE,
                    mybir.EngineType.Pool,
                    mybir.EngineType.SP,
                )
                with tc.For_i(0, loop_iters, hint_engines=engines):
                    body(nc, tc, sbuf, x, wqkv_t, wl_t, y, y2)
            else:
                body(nc, tc, sbuf, x, wqkv_t, wl_t, y, y2)

    nc.compile()
    return nc


_cached_nc = None


def kernel(x, Wq, Wk, Wv, Wlast, gamma):
    global _cached_nc
    x = np.ascontiguousarray(np.asarray(x, dtype=np.float32))
    B = x.shape[0]
    assert B == 8 and x.shape[1:] == (C, 64, 64)
    wqkv_t = np.ascontiguousarray(
        np.concatenate([Wq, Wk, Wv], axis=0).T.astype(np.float32)
    )
    wl_t = np.ascontiguousarray(
        (np.asarray(Wlast, np.float32) * np.float32(np.asarray(gamma)[0])).T
    )

    if _cached_nc is None:
        _cached_nc = build()
    nc = _cached_nc

    in_maps = [
        {
            "x": np.ascontiguousarray(x[b].reshape(C, L)),
            "wqkv_t": wqkv_t,
            "wl_t": wl_t,
        }
        for b in range(B)
    ]
    res = bass_utils.run_bass_kernel_spmd(nc, in_maps, core_ids=list(range(B)))
    lw = CHUNKS[-1][1]
    outs = []
    for b in range(B):
        yb = np.array(res.results[b]["y"]).reshape(C, L)
        yb[:, L - lw:] = res.results[b]["y2"].astype(np.float32)
        outs.append(yb.reshape(C, 64, 64))
    return np.stack(outs).astype(np.float32)



# revision 81
# speedup vs baseline: 1.0028x; 1.0005x over previous
"""Trainium2 Bass kernel for nn_AttentionLayer_47596827574368.

Reference computation (per batch sample b, B=8, C=768, H=W=64, L=4096, Cqk=Cv=96):
  Q = Wq @ X, K = Wk @ X, V = Wv @ X            (X = x[b] as [C, L])
  S = Q^T K   [L, L];  beta = softmax(S, axis=-1)
  O = beta @ V^T      [L, Cv]
  y = gamma * (Wlast @ O^T) + X                 [C, L]

Sharding: data-parallel over batch — one sample per NeuronCore (8 cores).

Device plan (per core):
  X streamed in (chunk, 512-col) pieces; Q/K c-major [96, 4096]; V^T as 32
  blocks [128(k), 97] (col 96 = ones -> softmax denominators ride along in
  the attnV matmul); scores computed transposed S^T[k, q] per 128-k block.
  Softmax uses a global-shift exp (C = est_max + 8 sampled from k-block 0;
  exact per-row max is unnecessary: softmax is shift-invariant and fp32 exp
  has huge dynamic-range headroom). Normalization is applied before the
  final projection; gamma is folded into Wlast on the host.

  The 16.7M-element exp of the score matrix is the ScalarE bottleneck
  (~1ns/element at 1.2GHz, vs the PE's 0.83ns/element of matmul alongside),
  so the PSUM score drain is split: ScalarE runs exact exp on ~2/3 of the
  512-wide units, VectorE runs a one-instruction Schraudolph approximation
  on the rest (uint16(A*s + B) bitcast as bf16, max rel err ~3.3%;
  fp32->uint16 conversion rounds and saturates negatives to 0, so
  deep-negative scores become +0.0). Numerator and denominator use the same
  approximate weights, so softmax stays a proper weighted average; measured
  end-to-end error is 8.6e-3 (budget 2e-2). GPSIMD/Pool cannot read PSUM
  on TRN2, so it only handles the SBUF-side reciprocal broadcasts.

  q columns are processed in chunks [512, 1024, 1024, 1024, 512]: the first
  512 chunk is interleaved with the projection phase, the last 512 chunk
  keeps the final drain short. Scores/exp/attnV run at 512-wide unit
  granularity through 6 single-bank PSUM score slots (up to 6 exps in
  flight across both drain engines); the attnV accumulation trails exp by a
  deep unit lag to ride out exp-latency spikes. Chunk normalization runs
  reciprocal on DVE, broadcast + multiply on Pool off a ScalarE-staged
  SBUF copy. (A Pool-side ones/denom divide simmed 0.2us faster but the
  gpsimd divide op fails the real device compile — TimelineSim does not
  validate op support, so HW-verify any new op type.)
  Each chunk's final projection + residual is spread through the
  next chunk's k loop (residual adds on DVE). The last chunk's tail is the
  critical serial path: it normalizes straight from PSUM in two 256-wide
  halves — both reciprocals issued back-to-back on DVE into disjoint
  slices of ONE rcp tile (separate single-buf ring allocations would
  serialize recip1 behind broadcast0's read), broadcasts on Pool, muls on
  DVE — so the Wlast projection matmuls start ~1.5us earlier than a
  full-width chain; the residual arrives via identity-matmul accumulates
  issued during the normalize latency; the drain is 3 ScalarE + 3 DVE
  bf16 copies feeding three paired DMAs on SP (each group one ScalarE +
  one DVE copy; bf16 halves the tail wire, and the 0.39% step is far
  under the error budget — the host converts back and stitches). Matmuls
  run in float32r (full PE rate). PSUM: proj phase 3+1+2 banks (+2 aux);
  main phase 6 score slots + 2 accumulator banks.

  Note for future tuning: the Tile framework list-schedules per engine by
  readiness (program order is only a tie-break), so "issue X later" code
  motion does NOT delay X — boundary ops like the ostage copy hoist into
  any engine idle slot as soon as their inputs are ready. A 1024-wide
  paired-exp variant (one exp per two 512 units; see kernel_pair.py) cuts
  ~17us of drain-engine busy time but loses ~7us to coarser score-slot
  recycling at chunk boundaries — net worse while the PE, not the drain
  engines, is the bottleneck (~91% busy).

  The last phase4 output block of chunk 3 is issued late (after the last
  chunk's attnV drain, identity-residual + ScalarE-copy form): its four
  matmuls fill the PE gap while the tail normalize chain runs, and its
  ~1.2us DVE add stays out of the tail window. The first of the last
  chunk's four trailing exps runs on DVE (tailpat 'vaaa') so the z-ring
  slots recycle without serializing behind ScalarE's backlog.

  The shift estimate samples k-block 0's first 256 q columns (32k scores
  — statistically ample, and the reduce_max sits on the chunk-0 exp
  critical path; halving it from 512 was worth ~170ns).

  Timeline-sim: 168921 ns/core (prior session: 171303, original baseline:
  198420); measured HW rel err 8.5e-3 (budget 2e-2). The exp engine
  schedules (vset {1,4}, midtail 'aava', tailpat 'vaaa', c0tail 'avav')
  came from exhaustive joint sweeps — re-sweep them all after ANY
  structural change; their optima shift and stale settings cost ~500ns.
"""

import numpy as np

import concourse.bass as bass
import concourse.tile as tile
import concourse.mybir as mybir
from concourse import bacc
from concourse import bass_utils
from concourse.masks import make_identity
from concourse import bass_isa

F32 = mybir.dt.float32
F32R = mybir.dt.float32r
BF16 = mybir.dt.bfloat16
U16 = mybir.dt.uint16
AF = mybir.ActivationFunctionType
AX = mybir.AxisListType
OP = mybir.AluOpType

C = 768          # input/output channels
CQ = 96          # qk/v channels
L = 4096         # H*W
KC = C // 128    # 6 contraction chunks
NKB = L // 128   # 32 k blocks
MARGIN = 8.0     # exp shift safety margin

# Schraudolph bf16-bits exp: exp(x) ~ bitcast_bf16(uint16(SCH_A*x + SCH_B)).
# Tuned on-device: fp32->uint16 converts round-to-nearest with saturation.
SCH_A = float(np.float32(128.0 / np.log(2.0)))
SCH_B = 16250.5

CHUNKS = [(0, 512), (512, 1024), (1536, 1024), (2560, 1024), (3584, 512)]
ET_BUFS = 12
S_BUFS = 6      # 512-wide score slots: exp latency tolerance / parallelism

# Schedule knobs (sweepable via sim_trace/sweep scripts):
CFG = {
    "lag": (8, 12, 4),       # attnV unit lag for (chunk0, mid, last)
    "vset": (1, 4),          # u%8 residues DVE takes in mid chunks
    "tailpat": "vaaa",       # engines of the last chunk's last 4 exps
    "bridge_eng": "av",      # bridge exp engines (pool-close race)
    "split_start": False,    # split the first x piece into 256-col halves
    "psum_recip": False,     # recip from PSUM denom row (worse in sim)
    "c0tail": "avav",        # engines of chunk 0's last 4 exps
    "midtail": "aava",       # engines of mid chunks' last 4 exps
    "est_w": 256,            # shift-estimate sample width (reduce_max cols)
}


def attnv_lag(ci):
    """attnV trails exp by this many 512-wide units. Chunk 0's drain gates
    the PSUM pool swap and the last chunk's drain gates the tail, so they
    may use shallower lags than the latency-tolerant mid chunks."""
    l0, lm, ll = CFG["lag"]
    if ci == 0:
        return l0
    if ci == len(CHUNKS) - 1:
        return ll
    return lm


def exp_engine(ci, u, n_units):
    """Which engine computes exp for 512-wide unit u (= kb*halves + h) of
    chunk ci. 'a' = ScalarE exact exp; 'v' = DVE Schraudolph approx (~1/3
    of units, validated end-to-end at ~7e-3 rel err).

    GPSIMD/Pool cannot read PSUM on TRN2 (BIR verifier rejects it), so only
    ScalarE and DVE can drain the score tiles. A chunk's first 8 units stay
    on ScalarE (DVE's queue holds the previous chunk's recip/mul then); two
    of the last 4 go to DVE so the trailing exps — which gate the next
    chunk's score slots — don't sit behind ScalarE's backlog."""
    if u >= n_units - 4:
        # trailing units gate the next chunk's score slots (or the tail's
        # z-ring); split them across both engines so neither serializes
        if ci == len(CHUNKS) - 1:
            return CFG.get("tailpat", "aaaa")[u % 4]
        if ci == 0:
            return CFG.get("c0tail", "vava")[u % 4]
        return CFG.get("midtail", "vava")[u % 4]
    if ci == 0:
        # proj phase: mostly ScalarE (it has slack at 512-wide tiles), with
        # a small DVE share so chunk-0's score slots never back up
        m, r, lo = CFG.get("c0v", (4, 1, 4))
        return 'v' if (u % m == r and u >= lo) else 'a'
    if u < 8:
        # alternate from the start: DVE only pays the previous chunk's
        # reciprocal now, so it can take every other early unit
        return CFG.get("head", "vavavava")[u]
    vs = CFG.get(f"vset{ci}") or CFG.get("vset", (2, 5, 7))
    if u % 8 in vs:
        return 'v'
    return 'a'


def pieces(w):
    # split a chunk width into matmul-sized pieces (<=512, >=256 so f32r
    # stays at full rate and no PSUM bank is crossed)
    out = []
    off = 0
    while w - off > 512:
        out.append((off, 512))
        off += 512
    out.append((off, w - off))
    return out


def body(nc, tc, sbuf, x, wqkv_t, wl_t, y, y2):
    # ---- persistent sbuf tiles -----------------------------------------
    # weights first (per-kc pieces so the first proj matmul starts early)
    w_sb = sbuf.tile([128, KC, 3 * CQ], F32R, tag="w")
    w_r = wqkv_t.rearrange("(ko ki) m -> ki ko m", ki=128).bitcast(F32R)
    # DMA issue costs ~650ns of sequencer time apiece, so startup spreads
    # issues across queues: weights on Pool's SWDGE, x on SP — the first
    # w slice (just Wq kc=0) and the first x piece reach the (shared,
    # serial) HWDGE back-to-back instead of ~1.3us apart; the identity
    # setup must come AFTER these issues (its Pool memsets would delay the
    # w SWDGE descriptor generation by ~1.5us -> +0.9us end-to-end)
    nc.gpsimd.dma_start(out=w_sb[:, 0, 0:CQ], in_=w_r[:, 0, 0:CQ])

    x_sb = sbuf.tile([128, KC, L], F32R, tag="x")
    x_r = x.rearrange("(ko ki) l -> ki ko l", ki=128).bitcast(F32R)
    # first piece split in half: the first projection matmuls run on
    # [*, 0:256] and start ~0.35us earlier (half the first-piece wire)
    if CFG.get("split_start", True):
        nc.sync.dma_start(out=x_sb[:, 0, 0:256], in_=x_r[:, 0, 0:256])
        nc.scalar.dma_start(out=w_sb[:, 0, CQ:], in_=w_r[:, 0, CQ:])
        nc.sync.dma_start(out=x_sb[:, 0, 256:512], in_=x_r[:, 0, 256:512])
    elif CFG.get("x0_swdge"):
        # first piece through the software DGE on Pool's queue: ~25ns seq
        # + ~1us fixed vs the HWDGE path's ~0.6+1.8us — lands ~0.8us sooner
        nc.gpsimd.dma_start(out=x_sb[:, 0, 0:512], in_=x_r[:, 0, 0:512])
        nc.scalar.dma_start(out=w_sb[:, 0, CQ:], in_=w_r[:, 0, CQ:])
    else:
        nc.sync.dma_start(out=x_sb[:, 0, 0:512], in_=x_r[:, 0, 0:512])
        nc.scalar.dma_start(out=w_sb[:, 0, CQ:], in_=w_r[:, 0, CQ:])
    # interleave the remaining weight pieces between group-0 x pieces in
    # demand order — weights queued up-front would push the x stream (the
    # projection pacer) back by ~3us
    if CFG.get("merge_w", False):
        # one merged DMA for w2..w5 frees three serial ~625ns HWDGE slots,
        # pulling every group-1/2 x piece earlier (they pace the PE around
        # 10-13us); w1 stays separate so kc=1's matmul isn't gated on the
        # whole merged wire
        nc.sync.dma_start(out=x_sb[:, 1, 0:512], in_=x_r[:, 1, 0:512])
        nc.scalar.dma_start(out=w_sb[:, 1, :], in_=w_r[:, 1, :])
        nc.sync.dma_start(out=x_sb[:, 2, 0:512], in_=x_r[:, 2, 0:512])
        nc.scalar.dma_start(out=w_sb[:, 2:, :], in_=w_r[:, 2:, :])
        for kc in range(3, KC):
            nc.sync.dma_start(out=x_sb[:, kc, 0:512], in_=x_r[:, kc, 0:512])
    else:
        for kc in range(1, KC):
            nc.sync.dma_start(out=x_sb[:, kc, 0:512], in_=x_r[:, kc, 0:512])
            nc.scalar.dma_start(out=w_sb[:, kc, :], in_=w_r[:, kc, :])
    wl_sb = sbuf.tile([CQ, C], F32R, tag="wl")
    # remaining groups in consumption order (wl after group 1 — first
    # needed ~60us in, by phase4 of chunk 0)
    for gp in range(1, 8):
        gs = slice(gp * 512, (gp + 1) * 512)
        for kc in range(KC):
            nc.sync.dma_start(out=x_sb[:, kc, gs], in_=x_r[:, kc, gs])
        if gp == CFG.get("wl_after", 1):
            nc.scalar.dma_start(out=wl_sb, in_=wl_t.bitcast(F32R))

    ident = sbuf.tile([128, 128], F32, tag="ident")
    make_identity(nc, ident)
    ident_bf = sbuf.tile([128, 128], BF16, tag="identbf")
    make_identity(nc, ident_bf)
    # f32r copy of the identity (the residual-add matmul needs an f32r
    # producer; a plain bitcast of the F32 tile fails BIR verification)
    ident_r = sbuf.tile([128, 128], F32R, tag="identr")
    nc.scalar.copy(ident_r, ident)

    q_sb = sbuf.tile([CQ, L], F32R, tag="q")
    k_sb = sbuf.tile([CQ, L], F32R, tag="k")
    v_sb = sbuf.tile([CQ, L], BF16, tag="vbig")
    vt_sb = sbuf.tile([128, NKB, CQ + 1], BF16, tag="vt")
    # ones column (f32r producer required: memset can't write f32r)
    nc.scalar.activation(
        out=vt_sb[:, :, CQ : CQ + 1].rearrange("p a b -> p (a b)"),
        in_=ident[:, 0:NKB],
        func=AF.Copy,
        bias=1.0,
        scale=0.0,
    )

    small = sbuf.tile([128, 16], F32, tag="small")
    m_row = small[:, 8:9]
    neg_c = small[:, 9:10]
    gmax_bc = small[:, 10:11]
    b_eff = small[:, 12:13]       # SCH_A * neg_c + SCH_B  (per partition)
    mt_sb = None if CFG.get("par_reduce", True) else \
        sbuf.tile([1, 128], F32, tag="rcp")

    attn_sb = sbuf.tile([CQ, L], F32R, tag="vbig", name="attn_sb")
    rcp_bc = sbuf.tile([CQ, 1024], F32, tag="rbc")
    y_r = y.rearrange("(ko ki) l -> ki ko l", ki=128)
    y2_r = y2.rearrange("(ko ki) l -> ki ko l", ki=128)

    def scores_mms(s_ps, kb, c0, w):
        for off, pw in pieces(w):
            nc.tensor.matmul(
                s_ps[:, off : off + pw],
                k_sb[:, kb * 128 : (kb + 1) * 128],
                q_sb[:, c0 + off : c0 + off + pw],
                start=True,
                stop=True,
            )

    def attnv_mm(out_ps, et, kb, off, pw):
        nc.tensor.matmul(
            out_ps[0 : CQ + 1, off : off + pw],
            vt_sb[:, kb, :],
            et[:, 0:pw],
            start=(kb == 0),
            stop=(kb == NKB - 1),
        )

    def exp_tile(ci, u, s_ps, pw, n_units=NKB * 2):
        """exp(s - C) into a bf16 et tile, on the engine exp_engine says."""
        et = sbuf.tile([128, 512], BF16, tag="et", bufs=ET_BUFS,
                       name=f"et_{ci}_{u}")[:, 0:pw]
        eng = exp_engine(ci, u, n_units)
        if eng == 'a':
            nc.scalar.activation(et, s_ps, AF.Exp, bias=neg_c, scale=1.0)
        else:
            nc.vector.tensor_scalar(et.bitcast(U16), s_ps, SCH_A, b_eff,
                                    OP.mult, OP.add)
        return et

    def normalize(ci, out_ps):
        #   attn[:, c0:c0+w] = out_ps[0:96] * (1 / out_ps[96])
        # Mid chunks read straight from PSUM (no staging copy): the
        # accumulator's banks are only needed again two chunks later, and
        # skipping the copy cuts the serial recip -> broadcast -> mul latency
        # at each chunk boundary. Chunk 0 lives in the ps_proj pool whose
        # close barrier gates all of chunk 1, so it stages through SBUF with
        # one fast ScalarE copy to release its banks immediately.
        c0, w = CHUNKS[ci]
        if ci == len(CHUNKS) - 1:
            # tail: DVE is otherwise idle; straight from PSUM is the
            # shortest chain before phase4
            rcp_sb = sbuf.tile([1, 1024], F32, tag="rcp",
                               name=f"rcp_{ci}")[:, 0:w]
            nc.vector.reciprocal(rcp_sb, out_ps[CQ : CQ + 1, 0:w])
            nc.gpsimd.partition_broadcast(rcp_bc[:, 0:w], rcp_sb)
            nc.vector.tensor_mul(attn_sb[:, c0 : c0 + w], out_ps[0:CQ, 0:w],
                                 rcp_bc[:, 0:w])
        else:
            # stage through SBUF (ScalarE) and multiply on Pool: DVE only
            # pays the reciprocal, so its exp share starts ~2us earlier at
            # each chunk boundary — where ScalarE alone can't keep the
            # score slots draining at the PE's pace. Also frees the
            # accumulator's banks early (obig has a single slot).
            # the denominator row rides in the staged copy, so the
            # reciprocal reads SBUF — for chunk 0 that takes it off the
            # ps_proj pool-close path (which gates all of chunk 1)
            ostage = sbuf.tile([CQ + 1, 1024], F32, tag="ostage", bufs=1,
                               name=f"ostage_{ci}")[:, 0:w]
            nc.scalar.copy(ostage, out_ps[0 : CQ + 1, 0:w])
            rcp_sb = sbuf.tile([1, 1024], F32, tag="rcp",
                               name=f"rcp_{ci}")[:, 0:w]
            nc.vector.reciprocal(rcp_sb, ostage[CQ : CQ + 1, :])
            nc.gpsimd.partition_broadcast(rcp_bc[:, 0:w], rcp_sb)
            nc.gpsimd.tensor_mul(attn_sb[:, c0 : c0 + w], ostage[0:CQ, :],
                                 rcp_bc[:, 0:w])

    norm_tiles = {}

    def normalize_half(ci, out_ps, h):
        # Mid-chunk normalize, one 512-wide half at a time. Half 0 is
        # issued at the chunk's end; half 1 from inside the NEXT chunk's
        # k loop, a few units in: both halves are ready at the boundary,
        # so only priority (issue order) decides what the list scheduler
        # hoists into the boundary — deferring half 1 puts the next
        # chunk's first exps ahead of it in the ready-queue tie-breaks,
        # halving the boundary-resident ostage/recip work.
        c0, w = CHUNKS[ci]
        hw2 = w // 2
        hs = slice(h * hw2, (h + 1) * hw2)
        if h == 0:
            norm_tiles[ci] = (
                sbuf.tile([CQ + 1, 1024], F32, tag="ostage", bufs=1,
                          name=f"ostage_{ci}"),
                sbuf.tile([1, 1024], F32, tag="rcp", name=f"rcp_{ci}"),
            )
        ostage, rcp_sb = norm_tiles[ci]
        nc.scalar.copy(ostage[:, hs], out_ps[0 : CQ + 1, hs])
        nc.vector.reciprocal(rcp_sb[:, hs], ostage[CQ : CQ + 1, hs])
        nc.gpsimd.partition_broadcast(rcp_bc[:, hs], rcp_sb[:, hs])
        nc.gpsimd.tensor_mul(attn_sb[:, c0 + h * hw2 : c0 + (h + 1) * hw2],
                             ostage[0:CQ, hs], rcp_bc[:, hs])

    def phase4_unit(ps_pool, ci, oc, spread=True):
        # final projection + residual for one 128-row output chunk
        c0, w = CHUNKS[ci]
        if spread:
            # z halves borrow score slots (exp lookahead briefly 6 -> 4);
            # each half's add fires as soon as its matmul lands, DVE taking
            # one half and Pool the other
            y_sb = sbuf.tile([128, 1024], F32, tag="y", bufs=3,
                             name=f"y_sb_{ci}_{oc}")[:, 0:w]
            for g, (off, pw) in enumerate(pieces(w)):
                z_ps = ps_pool.tile([128, 512], F32, tag="s", bufs=S_BUFS,
                                    name=f"z_ps_{ci}_{oc}_{g}")[:, 0:pw]
                nc.tensor.matmul(
                    z_ps,
                    wl_sb[:, oc * 128 : (oc + 1) * 128],
                    attn_sb[:, c0 + off : c0 + off + pw],
                    start=True,
                    stop=True,
                )
                nc.vector.tensor_add(y_sb[:, off : off + pw], z_ps,
                               x_sb[:, oc, c0 + off : c0 + off + pw].bitcast(F32))
            nc.sync.dma_start(out=y_r[:, oc, slice(c0, c0 + w)], in_=y_sb)
            return
        # late-issued unit: residual via identity-matmul accumulate and a
        # ScalarE copy — its matmuls fill the last chunk's normalize-wait
        # PE gap, and it keeps the ~1.2us DVE add out of the tail window
        y_sb = sbuf.tile([128, 1024], F32, tag="y", bufs=3,
                         name=f"y_sb_{ci}_{oc}")[:, 0:w]
        for g, (off, pw) in enumerate(pieces(w)):
            zp = ps_pool.tile([128, 512], F32, tag="s", bufs=S_BUFS,
                              name=f"z_late_{ci}_{oc}_{g}")[:, 0:pw]
            nc.tensor.matmul(zp, ident_r, x_sb[:, oc, c0 + off : c0 + off + pw],
                             start=True, stop=False)
            nc.tensor.matmul(
                zp,
                wl_sb[:, oc * 128 : (oc + 1) * 128],
                attn_sb[:, c0 + off : c0 + off + pw],
                start=False,
                stop=True,
            )
            nc.scalar.copy(y_sb[:, off : off + pw], zp)
        nc.sync.dma_start(out=y_r[:, oc, slice(c0, c0 + w)], in_=y_sb)

    def phase4_last(ps_pool, ci, out_ps):
        # Last chunk: normalize from PSUM in two 256-wide halves (both
        # reciprocals on DVE up front, broadcasts on Pool, then the muls)
        # so the first projection matmuls start ~1us earlier than a
        # full-width chain; residual via identity-matmul accumulates
        # issued during the normalize latency; PSUM->SBUF bf16 copies
        # split 3 ScalarE / 3 DVE feeding three paired DMAs on SP (each
        # group one ScalarE + one DVE copy, so the last DMA issues as
        # early as either engine allows).
        c0, w = CHUNKS[ci]
        hw = w // 2
        zs = []
        for oc in range(KC):
            z_ps = ps_pool.tile([128, 512], F32, tag="s", bufs=S_BUFS,
                                name=f"z_ps_{ci}_{oc}")[:, 0:w]
            nc.tensor.matmul(
                z_ps,
                ident_r,
                x_sb[:, oc, c0 : c0 + w],
                start=True,
                stop=False,
            )
            zs.append(z_ps)
        # one rcp tile sliced per half: separate ring allocations would
        # serialize recip1 behind bcast0's read of the single-buf slot
        rcp2 = sbuf.tile([1, 1024], F32, tag="rcp", name=f"rcp_{ci}")
        for h in range(2):
            hs = slice(h * hw, (h + 1) * hw)
            nc.vector.reciprocal(rcp2[:, hs], out_ps[CQ : CQ + 1, hs])
            nc.gpsimd.partition_broadcast(rcp_bc[:, hs], rcp2[:, hs])
        for h in range(2):
            hs = slice(h * hw, (h + 1) * hw)
            nc.vector.tensor_mul(attn_sb[:, c0 + h * hw : c0 + (h + 1) * hw],
                                 out_ps[0:CQ, hs], rcp_bc[:, hs])
            for oc in range(KC):
                nc.tensor.matmul(
                    zs[oc][:, hs],
                    wl_sb[:, oc * 128 : (oc + 1) * 128],
                    attn_sb[:, c0 + h * hw : c0 + (h + 1) * hw],
                    start=False,
                    stop=(h == 1),
                )
        ysg = [sbuf.tile([128, 2, 512], BF16, tag=f"ylast{g}", bufs=1,
                         name=f"y_last_{g}") for g in range(3)]
        for g in range(3):
            nc.scalar.copy(ysg[g][:, 0, :], zs[2 * g])
            nc.vector.tensor_copy(ysg[g][:, 1, :], zs[2 * g + 1])
        # SP queue only: a ScalarE-issued DMA would block ScalarE's in-order
        # queue and delay the remaining copies
        for g in range(3):
            nc.sync.dma_start(out=y2_r[:, 2 * g : 2 * g + 2, :], in_=ysg[g])

    # ---- phase 1 + attention chunk 0 (512 wide), interleaved ------------
    # projections run in 512-column groups; as each group's K/V land, the
    # corresponding k-blocks of chunk 0 are scored/exp'd/accumulated.
    with (
        tc.tile_pool(name="ps_proj", bufs=1, space="PSUM") as ps_proj,
        tc.tile_pool(name="ps_aux", bufs=2, space="PSUM") as ps_aux,
    ):
        out0_ps = ps_proj.tile([128, 512], F32, tag="o0", name="out0_ps")
        # PE p-state warmup: the clock runs at half rate until 3us of
        # CONTINUOUS busy, and the first x piece only lands ~3.9us in.
        # Dummy identity transposes (no readers, recycled ps_aux ring)
        # keep the PE busy from ~0.3us so the ramp completes before the
        # first projection matmul — which then runs at the full 2.4GHz.
        for wu in range(CFG.get("warmup", 0)):
            wu_ps = ps_aux.tile([128, 128], F32, tag="sm", name=f"wu_{wu}")
            nc.tensor.transpose(wu_ps, ident, ident)
        pend_attnv = []  # attnV lag FIFO so PE never waits on exp in-order
        for gp in range(8):
            gs = slice(gp * 512, (gp + 1) * 512)
            tiles = [
                ps_proj.tile([CQ, 512], F32, tag=f"proj{t}", name=f"p_ps_{t}_{gp}")
                for t in range(3)
            ]
            for kc in range(KC):
                for t in range(3):
                    if gp == 0 and kc == 0 and CFG.get("split_start", True):
                        # x arrives in two 256-col halves; start on the first
                        for ho in (0, 256):
                            nc.tensor.matmul(
                                tiles[t][:, ho : ho + 256],
                                w_sb[:, 0, t * CQ : (t + 1) * CQ],
                                x_sb[:, 0, ho : ho + 256],
                                start=True,
                                stop=False,
                                skip_group_check=True,
                            )
                        continue
                    nc.tensor.matmul(
                        tiles[t],
                        w_sb[:, kc, t * CQ : (t + 1) * CQ],
                        x_sb[:, kc, gs],
                        start=(kc == 0),
                        stop=(kc == KC - 1),
                        skip_group_check=(gp == 0),
                    )
            for t, dst in ((0, q_sb), (1, k_sb), (2, v_sb)):
                if t == 1:
                    nc.vector.tensor_copy(dst[:, gs], tiles[t])
                else:
                    nc.scalar.copy(dst[:, gs], tiles[t])

            # chunk-0 attention for this group's 4 k-blocks
            for kb in range(4 * gp, 4 * gp + 4):
                s_ps = ps_proj.tile([128, 512], F32, tag="s0", bufs=2,
                                    name=f"s_ps_0_{kb}")
                scores_mms(s_ps, kb, 0, 512)
                if kb == 0:
                    # shift estimate from these 32k scores (statistically
                    # ample for a shift that merely has to land within
                    # ~+-80 of the true max). This chain gates every
                    # chunk-0 exp, so it must be SHORT.
                    nc.vector.reduce_max(m_row, s_ps[:, 0:CFG.get('est_w', 512)], axis=AX.X)
                    if CFG.get("par_reduce", True):
                        # one Pool all-reduce replaces the four-link
                        # transpose/copy/reduce/broadcast partition chain
                        nc.gpsimd.partition_all_reduce(
                            gmax_bc, m_row, 128, bass_isa.ReduceOp.max)
                    else:
                        mt_ps = ps_aux.tile([1, 128], F32, tag="sm")
                        nc.tensor.transpose(mt_ps, m_row, ident)
                        nc.vector.tensor_copy(mt_sb[:, 0:128], mt_ps)
                        nc.vector.reduce_max(small[0:1, 11:12],
                                             mt_sb[:, 0:128], axis=AX.X)
                        nc.gpsimd.partition_broadcast(gmax_bc,
                                                      small[0:1, 11:12])
                    # neg_c = -(gmax + MARGIN)
                    nc.scalar.activation(neg_c, gmax_bc, AF.Copy,
                                         bias=-MARGIN, scale=-1.0)
                    # b_eff = SCH_A * neg_c + SCH_B (for the approx engines)
                    nc.vector.tensor_scalar(b_eff, neg_c, SCH_A, SCH_B,
                                            OP.mult, OP.add)
                et = exp_tile(0, kb, s_ps, 512, n_units=NKB)
                if len(pend_attnv) >= attnv_lag(0):
                    attnv_mm(out0_ps, *pend_attnv.pop(0))
                pend_attnv.append((et, kb, 0, 512))
            # V -> V^T transposes for this group's 4 l-blocks (the last
            # group's copies optionally on ScalarE so DVE is free for the
            # bridge exps that gate the pool swap)
            for lb in range(4 * gp, 4 * gp + 4):
                t_ps = ps_aux.tile([128, CQ], BF16, tag="sm", name=f"t_ps_{lb}")
                nc.tensor.transpose(
                    t_ps, v_sb[:, lb * 128 : (lb + 1) * 128], ident_bf[0:CQ, 0:CQ]
                )
                if gp == 7 and CFG.get("vt7_scalar"):
                    nc.scalar.copy(vt_sb[:, lb, 0:CQ], t_ps)
                else:
                    nc.vector.tensor_copy(vt_sb[:, lb, 0:CQ], t_ps)

        for pa in pend_attnv:
            attnv_mm(out0_ps, *pa)
        # chunk-0 normalize first: its ScalarE staging copy releases the
        # out0 banks so the pool-close barrier (gating all of chunk 1) isn't
        # stuck behind the bridge exps
        normalize(0, out0_ps)
        # bridge: score+exp chunk-1's k-block 0 in this pool's slots so
        # ScalarE never idles across the PSUM pool swap
        bridge_units = []
        for bu in range(2 * CFG.get("bridge_kb", 1)):
            kb, h = bu // 2, bu % 2
            sb_ps = ps_proj.tile([128, 512], F32, tag="s0", bufs=2,
                                 name=f"sb_ps_{bu}")
            nc.tensor.matmul(
                sb_ps, k_sb[:, kb * 128 : (kb + 1) * 128],
                q_sb[:, 512 + h * 512 : 512 + (h + 1) * 512],
                start=True, stop=True,
            )
            bet = sbuf.tile([128, 512], BF16, tag="et", bufs=ET_BUFS,
                            name=f"et_1_0_{bu}")
            # engine choice: the pool close (gating all of chunk 1) waits on
            # these exps' PSUM reads, racing the other engine's backlog
            if CFG.get("bridge_eng", "vv")[bu % len(CFG.get("bridge_eng", "vv"))] == 'v':
                nc.vector.tensor_scalar(bet.bitcast(U16), sb_ps, SCH_A, b_eff,
                                        OP.mult, OP.add)
            else:
                nc.scalar.activation(bet, sb_ps, AF.Exp, bias=neg_c, scale=1.0)
            bridge_units.append((bet, kb, h * 512, 512))

    # ---- attention chunks 1..4 ------------------------------------------
    with tc.tile_pool(name="ps_attn", bufs=1, space="PSUM") as ps_attn:
        prev_ps = [None]   # previous chunk's accumulator awaiting half-1
        for ci in range(1, len(CHUNKS)):
            c0, w = CHUNKS[ci]
            out_ps = ps_attn.tile(
                [128, 1024], F32, tag="obig", bufs=1, name=f"out_ps_{ci}"
            )
            # attnV trails exp by ATTNV_LAG 512-wide units; 4 s_ps slots let
            # up to 4 exps run concurrently across ScalarE/DVE/Pool
            pend = list(bridge_units) if ci == 1 else []
            nh = len(pieces(w))
            for kb in range(CFG.get("bridge_kb", 1) if ci == 1 else 0, NKB):
                for h, (off, pw) in enumerate(pieces(w)):
                    u = kb * nh + h
                    s_ps = ps_attn.tile(
                        [128, 512], F32, tag="s", bufs=S_BUFS,
                        name=f"s_ps_{ci}_{u}"
                    )[:, 0:pw]
                    nc.tensor.matmul(
                        s_ps,
                        k_sb[:, kb * 128 : (kb + 1) * 128],
                        q_sb[:, c0 + off : c0 + off + pw],
                        start=True,
                        stop=True,
                    )
                    et = exp_tile(ci, u, s_ps, pw, n_units=NKB * nh)
                    if len(pend) >= attnv_lag(ci):
                        attnv_mm(out_ps, *pend.pop(0))
                    pend.append((et, kb, off, pw))
                    if u == CFG.get("h1_at", 3) and ci >= 2 and \
                            CFG.get("split_norm", False):
                        normalize_half(ci - 1, prev_ps[0], 1)
                # spread the previous chunk's phase 4 through this chunk's
                # k loop, starting at kb=8 so the previous chunk's normalize
                # chain (which the z matmuls depend on) has finished — PE is
                # in-order, so an early-enqueued z matmul would stall scores
                last = ci == len(CHUNKS) - 1
                sp0, step = CFG.get("sp_last", (16, 3)) if last else CFG.get("sp_mid", (12, 3))
                # all spread units must fit inside this chunk's kb range — a
                # unit past kb=31 would silently drop an output block
                nsp = KC - 1 if (last and CFG.get("late6", True)) else KC
                assert sp0 + step * (nsp - 1) < NKB
                if (kb - sp0) % step == 0 and sp0 <= kb < sp0 + step * nsp:
                    phase4_unit(ps_attn, ci - 1, (kb - sp0) // step)
            for pe in pend:
                attnv_mm(out_ps, *pe)
            if ci == len(CHUNKS) - 1 and CFG.get("late6", True):
                # chunk-3's last output block, issued after the drain: its
                # matmuls fill the PE gap while the tail normalize runs
                phase4_unit(ps_attn, ci - 1, KC - 1, spread=False)
            if ci < len(CHUNKS) - 1:
                if CFG.get("split_norm", False):
                    normalize_half(ci, out_ps, 0)
                    prev_ps[0] = out_ps
                else:
                    normalize(ci, out_ps)

        # last chunk's normalize halves + phase 4
        phase4_last(ps_attn, len(CHUNKS) - 1, out_ps)


def build(loop_iters=1):
    nc = bacc.Bacc("TRN2", target_bir_lowering=False, debug=False, num_devices=8)
    x = nc.dram_tensor("x", [C, L], F32, kind="ExternalInput").ap()
    wqkv_t = nc.dram_tensor("wqkv_t", [C, 3 * CQ], F32, kind="ExternalInput").ap()
    wl_t = nc.dram_tensor("wl_t", [CQ, C], F32, kind="ExternalInput").ap()
    y = nc.dram_tensor("y", [C, L], F32, kind="ExternalOutput").ap()
    # last q-chunk's output in bf16: halves the tail's DMA wire time; the
    # 0.39% bf16 step is well under the error budget (host converts back)
    y2 = nc.dram_tensor("y2", [C, CHUNKS[-1][1]], mybir.dt.bfloat16,
                        kind="ExternalOutput").ap()

    with tile.TileContext(nc) as tc:
        with tc.tile_pool(name="sbuf", bufs=1) as sbuf:
            if loop_iters > 1:
                engines = (
                    mybir.EngineType.PE,
                    mybir.EngineType.Activation,
                    mybir.EngineType.DVE,
                    mybir.EngineType.Pool,
                    mybir.EngineType.SP,
                )
                with tc.For_i(0, loop_iters, hint_engines=engines):
                    body(nc, tc, sbuf, x, wqkv_t, wl_t, y, y2)
            else:
                body(nc, tc, sbuf, x, wqkv_t, wl_t, y, y2)

    nc.compile()
    return nc


_cached_nc = None


def kernel(x, Wq, Wk, Wv, Wlast, gamma):
    global _cached_nc
    x = np.ascontiguousarray(np.asarray(x, dtype=np.float32))
    B = x.shape[0]
    assert B == 8 and x.shape[1:] == (C, 64, 64)
    wqkv_t = np.ascontiguousarray(
        np.concatenate([Wq, Wk, Wv], axis=0).T.astype(np.float32)
    )
    wl_t = np.ascontiguousarray(
        (np.asarray(Wlast, np.float32) * np.float32(np.asarray(gamma)[0])).T
    )

    if _cached_nc is None:
        _cached_nc = build()
    nc = _cached_nc

    in_maps = [
        {
            "x": np.ascontiguousarray(x[b].reshape(C, L)),
            "wqkv_t": wqkv_t,
            "wl_t": wl_t,
        }
        for b in range(B)
    ]
    res = bass_utils.run_bass_kernel_spmd(nc, in_maps, core_ids=list(range(B)))
    lw = CHUNKS[-1][1]
    outs = []
    for b in range(B):
        yb = np.array(res.results[b]["y"]).reshape(C, L)
        yb[:, L - lw:] = res.results[b]["y2"].astype(np.float32)
        outs.append(yb.reshape(C, 64, 64))
    return np.stack(outs).astype(np.float32)



# revision 85
# speedup vs baseline: 1.0029x; 1.0001x over previous
"""Trainium2 Bass kernel for nn_AttentionLayer_47596827574368.

Reference computation (per batch sample b, B=8, C=768, H=W=64, L=4096, Cqk=Cv=96):
  Q = Wq @ X, K = Wk @ X, V = Wv @ X            (X = x[b] as [C, L])
  S = Q^T K   [L, L];  beta = softmax(S, axis=-1)
  O = beta @ V^T      [L, Cv]
  y = gamma * (Wlast @ O^T) + X                 [C, L]

Sharding: data-parallel over batch — one sample per NeuronCore (8 cores).

Device plan (per core):
  X streamed in (chunk, 512-col) pieces; Q/K c-major [96, 4096]; V^T as 32
  blocks [128(k), 97] (col 96 = ones -> softmax denominators ride along in
  the attnV matmul); scores computed transposed S^T[k, q] per 128-k block.
  Softmax uses a global-shift exp (C = est_max + 8 sampled from k-block 0;
  exact per-row max is unnecessary: softmax is shift-invariant and fp32 exp
  has huge dynamic-range headroom). Normalization is applied before the
  final projection; gamma is folded into Wlast on the host.

  The 16.7M-element exp of the score matrix is the ScalarE bottleneck
  (~1ns/element at 1.2GHz, vs the PE's 0.83ns/element of matmul alongside),
  so the PSUM score drain is split: ScalarE runs exact exp on ~2/3 of the
  512-wide units, VectorE runs a one-instruction Schraudolph approximation
  on the rest (uint16(A*s + B) bitcast as bf16, max rel err ~3.3%;
  fp32->uint16 conversion rounds and saturates negatives to 0, so
  deep-negative scores become +0.0). Numerator and denominator use the same
  approximate weights, so softmax stays a proper weighted average; measured
  end-to-end error is 8.6e-3 (budget 2e-2). GPSIMD/Pool cannot read PSUM
  on TRN2, so it only handles the SBUF-side reciprocal broadcasts.

  q columns are processed in chunks [512, 1024, 1024, 1024, 512]: the first
  512 chunk is interleaved with the projection phase, the last 512 chunk
  keeps the final drain short. Scores/exp/attnV run at 512-wide unit
  granularity through 6 single-bank PSUM score slots (up to 6 exps in
  flight across both drain engines); the attnV accumulation trails exp by a
  deep unit lag to ride out exp-latency spikes. Chunk normalization runs
  reciprocal on DVE, broadcast + multiply on Pool off a ScalarE-staged
  SBUF copy. (A Pool-side ones/denom divide simmed 0.2us faster but the
  gpsimd divide op fails the real device compile — TimelineSim does not
  validate op support, so HW-verify any new op type.)
  Each chunk's final projection + residual is spread through the
  next chunk's k loop (residual adds on DVE). The last chunk's tail is the
  critical serial path: it normalizes straight from PSUM in two 256-wide
  halves — both reciprocals issued back-to-back on DVE into disjoint
  slices of ONE rcp tile (separate single-buf ring allocations would
  serialize recip1 behind broadcast0's read), broadcasts on Pool, muls on
  DVE — so the Wlast projection matmuls start ~1.5us earlier than a
  full-width chain; the residual arrives via identity-matmul accumulates
  issued during the normalize latency; the drain is 3 ScalarE + 3 DVE
  bf16 copies feeding three paired DMAs on SP (each group one ScalarE +
  one DVE copy; bf16 halves the tail wire, and the 0.39% step is far
  under the error budget — the host converts back and stitches). Matmuls
  run in float32r (full PE rate). PSUM: proj phase 3+1+2 banks (+2 aux);
  main phase 6 score slots + 2 accumulator banks.

  Note for future tuning: the Tile framework list-schedules per engine by
  readiness (program order is only a tie-break), so "issue X later" code
  motion does NOT delay X — boundary ops like the ostage copy hoist into
  any engine idle slot as soon as their inputs are ready. A 1024-wide
  paired-exp variant (one exp per two 512 units; see kernel_pair.py) cuts
  ~17us of drain-engine busy time but loses ~7us to coarser score-slot
  recycling at chunk boundaries — net worse while the PE, not the drain
  engines, is the bottleneck (~91% busy).

  The last phase4 output block of chunk 3 is issued late (after the last
  chunk's attnV drain, identity-residual + ScalarE-copy form): its four
  matmuls fill the PE gap while the tail normalize chain runs, and its
  ~1.2us DVE add stays out of the tail window. The first of the last
  chunk's four trailing exps runs on DVE (tailpat 'vaaa') so the z-ring
  slots recycle without serializing behind ScalarE's backlog.

  The shift estimate samples k-block 0's first 128 q columns (16k scores
  — statistically ample, and the reduce_max sits on the chunk-0 exp
  critical path; halving it from 512 was worth ~170ns).

  The estimate's cross-partition max uses one Pool partition_all_reduce
  (HW-verified) instead of a four-link transpose/copy/reduce/broadcast
  chain.

  Timeline-sim: 168812 ns/core (prior session: 171303, original baseline:
  198420); measured HW rel err 8.5e-3 (budget 2e-2). The exp engine
  schedules (vset {1,4}, midtail 'aava', tailpat 'vaaa', c0tail 'avav')
  came from exhaustive joint sweeps — re-sweep them all after ANY
  structural change; their optima shift and stale settings cost ~500ns.
"""

import numpy as np

import concourse.bass as bass
import concourse.tile as tile
import concourse.mybir as mybir
from concourse import bacc
from concourse import bass_utils
from concourse.masks import make_identity
from concourse import bass_isa

F32 = mybir.dt.float32
F32R = mybir.dt.float32r
BF16 = mybir.dt.bfloat16
U16 = mybir.dt.uint16
AF = mybir.ActivationFunctionType
AX = mybir.AxisListType
OP = mybir.AluOpType

C = 768          # input/output channels
CQ = 96          # qk/v channels
L = 4096         # H*W
KC = C // 128    # 6 contraction chunks
NKB = L // 128   # 32 k blocks
MARGIN = 8.0     # exp shift safety margin

# Schraudolph bf16-bits exp: exp(x) ~ bitcast_bf16(uint16(SCH_A*x + SCH_B)).
# Tuned on-device: fp32->uint16 converts round-to-nearest with saturation.
SCH_A = float(np.float32(128.0 / np.log(2.0)))
SCH_B = 16250.5

CHUNKS = [(0, 512), (512, 1024), (1536, 1024), (2560, 1024), (3584, 512)]
ET_BUFS = 12
S_BUFS = 6      # 512-wide score slots: exp latency tolerance / parallelism

# Schedule knobs (sweepable via sim_trace/sweep scripts):
CFG = {
    "lag": (8, 12, 4),       # attnV unit lag for (chunk0, mid, last)
    "vset": (1, 4),          # u%8 residues DVE takes in mid chunks
    "tailpat": "vaaa",       # engines of the last chunk's last 4 exps
    "bridge_eng": "av",      # bridge exp engines (pool-close race)
    "split_start": False,    # split the first x piece into 256-col halves
    "psum_recip": False,     # recip from PSUM denom row (worse in sim)
    "c0tail": "avav",        # engines of chunk 0's last 4 exps
    "midtail": "aava",       # engines of mid chunks' last 4 exps
    "est_w": 128,            # shift-estimate sample width (reduce_max cols)
}


def attnv_lag(ci):
    """attnV trails exp by this many 512-wide units. Chunk 0's drain gates
    the PSUM pool swap and the last chunk's drain gates the tail, so they
    may use shallower lags than the latency-tolerant mid chunks."""
    l0, lm, ll = CFG["lag"]
    if ci == 0:
        return l0
    if ci == len(CHUNKS) - 1:
        return ll
    return lm


def exp_engine(ci, u, n_units):
    """Which engine computes exp for 512-wide unit u (= kb*halves + h) of
    chunk ci. 'a' = ScalarE exact exp; 'v' = DVE Schraudolph approx (~1/3
    of units, validated end-to-end at ~7e-3 rel err).

    GPSIMD/Pool cannot read PSUM on TRN2 (BIR verifier rejects it), so only
    ScalarE and DVE can drain the score tiles. A chunk's first 8 units stay
    on ScalarE (DVE's queue holds the previous chunk's recip/mul then); two
    of the last 4 go to DVE so the trailing exps — which gate the next
    chunk's score slots — don't sit behind ScalarE's backlog."""
    if u >= n_units - 4:
        # trailing units gate the next chunk's score slots (or the tail's
        # z-ring); split them across both engines so neither serializes
        if ci == len(CHUNKS) - 1:
            return CFG.get("tailpat", "aaaa")[u % 4]
        if ci == 0:
            return CFG.get("c0tail", "vava")[u % 4]
        return CFG.get("midtail", "vava")[u % 4]
    if ci == 0:
        # proj phase: mostly ScalarE (it has slack at 512-wide tiles), with
        # a small DVE share so chunk-0's score slots never back up
        m, r, lo = CFG.get("c0v", (4, 1, 4))
        return 'v' if (u % m == r and u >= lo) else 'a'
    if u < 8:
        # alternate from the start: DVE only pays the previous chunk's
        # reciprocal now, so it can take every other early unit
        return CFG.get("head", "vavavava")[u]
    vs = CFG.get(f"vset{ci}") or CFG.get("vset", (2, 5, 7))
    if u % 8 in vs:
        return 'v'
    return 'a'


def pieces(w):
    # split a chunk width into matmul-sized pieces (<=512, >=256 so f32r
    # stays at full rate and no PSUM bank is crossed)
    out = []
    off = 0
    while w - off > 512:
        out.append((off, 512))
        off += 512
    out.append((off, w - off))
    return out


def body(nc, tc, sbuf, x, wqkv_t, wl_t, y, y2):
    # ---- persistent sbuf tiles -----------------------------------------
    # weights first (per-kc pieces so the first proj matmul starts early)
    w_sb = sbuf.tile([128, KC, 3 * CQ], F32R, tag="w")
    w_r = wqkv_t.rearrange("(ko ki) m -> ki ko m", ki=128).bitcast(F32R)
    # DMA issue costs ~650ns of sequencer time apiece, so startup spreads
    # issues across queues: weights on Pool's SWDGE, x on SP — the first
    # w slice (just Wq kc=0) and the first x piece reach the (shared,
    # serial) HWDGE back-to-back instead of ~1.3us apart; the identity
    # setup must come AFTER these issues (its Pool memsets would delay the
    # w SWDGE descriptor generation by ~1.5us -> +0.9us end-to-end)
    nc.gpsimd.dma_start(out=w_sb[:, 0, 0:CQ], in_=w_r[:, 0, 0:CQ])

    x_sb = sbuf.tile([128, KC, L], F32R, tag="x")
    x_r = x.rearrange("(ko ki) l -> ki ko l", ki=128).bitcast(F32R)
    # first piece split in half: the first projection matmuls run on
    # [*, 0:256] and start ~0.35us earlier (half the first-piece wire)
    if CFG.get("split_start", True):
        nc.sync.dma_start(out=x_sb[:, 0, 0:256], in_=x_r[:, 0, 0:256])
        nc.scalar.dma_start(out=w_sb[:, 0, CQ:], in_=w_r[:, 0, CQ:])
        nc.sync.dma_start(out=x_sb[:, 0, 256:512], in_=x_r[:, 0, 256:512])
    elif CFG.get("x0_swdge"):
        # first piece through the software DGE on Pool's queue: ~25ns seq
        # + ~1us fixed vs the HWDGE path's ~0.6+1.8us — lands ~0.8us sooner
        nc.gpsimd.dma_start(out=x_sb[:, 0, 0:512], in_=x_r[:, 0, 0:512])
        nc.scalar.dma_start(out=w_sb[:, 0, CQ:], in_=w_r[:, 0, CQ:])
    else:
        nc.sync.dma_start(out=x_sb[:, 0, 0:512], in_=x_r[:, 0, 0:512])
        nc.scalar.dma_start(out=w_sb[:, 0, CQ:], in_=w_r[:, 0, CQ:])
    # interleave the remaining weight pieces between group-0 x pieces in
    # demand order — weights queued up-front would push the x stream (the
    # projection pacer) back by ~3us
    if CFG.get("merge_w", False):
        # one merged DMA for w2..w5 frees three serial ~625ns HWDGE slots,
        # pulling every group-1/2 x piece earlier (they pace the PE around
        # 10-13us); w1 stays separate so kc=1's matmul isn't gated on the
        # whole merged wire
        nc.sync.dma_start(out=x_sb[:, 1, 0:512], in_=x_r[:, 1, 0:512])
        nc.scalar.dma_start(out=w_sb[:, 1, :], in_=w_r[:, 1, :])
        nc.sync.dma_start(out=x_sb[:, 2, 0:512], in_=x_r[:, 2, 0:512])
        nc.scalar.dma_start(out=w_sb[:, 2:, :], in_=w_r[:, 2:, :])
        for kc in range(3, KC):
            nc.sync.dma_start(out=x_sb[:, kc, 0:512], in_=x_r[:, kc, 0:512])
    else:
        for kc in range(1, KC):
            nc.sync.dma_start(out=x_sb[:, kc, 0:512], in_=x_r[:, kc, 0:512])
            nc.scalar.dma_start(out=w_sb[:, kc, :], in_=w_r[:, kc, :])
    wl_sb = sbuf.tile([CQ, C], F32R, tag="wl")
    # remaining groups in consumption order (wl after group 1 — first
    # needed ~60us in, by phase4 of chunk 0)
    for gp in range(1, 8):
        gs = slice(gp * 512, (gp + 1) * 512)
        for kc in range(KC):
            nc.sync.dma_start(out=x_sb[:, kc, gs], in_=x_r[:, kc, gs])
        if gp == CFG.get("wl_after", 1):
            nc.scalar.dma_start(out=wl_sb, in_=wl_t.bitcast(F32R))

    ident = sbuf.tile([128, 128], F32, tag="ident")
    make_identity(nc, ident)
    ident_bf = sbuf.tile([128, 128], BF16, tag="identbf")
    make_identity(nc, ident_bf)
    # f32r copy of the identity (the residual-add matmul needs an f32r
    # producer; a plain bitcast of the F32 tile fails BIR verification)
    ident_r = sbuf.tile([128, 128], F32R, tag="identr")
    nc.scalar.copy(ident_r, ident)

    q_sb = sbuf.tile([CQ, L], F32R, tag="q")
    k_sb = sbuf.tile([CQ, L], F32R, tag="k")
    v_sb = sbuf.tile([CQ, L], BF16, tag="vbig")
    vt_sb = sbuf.tile([128, NKB, CQ + 1], BF16, tag="vt")
    # ones column (f32r producer required: memset can't write f32r)
    nc.scalar.activation(
        out=vt_sb[:, :, CQ : CQ + 1].rearrange("p a b -> p (a b)"),
        in_=ident[:, 0:NKB],
        func=AF.Copy,
        bias=1.0,
        scale=0.0,
    )

    small = sbuf.tile([128, 16], F32, tag="small")
    m_row = small[:, 8:9]
    neg_c = small[:, 9:10]
    gmax_bc = small[:, 10:11]
    b_eff = small[:, 12:13]       # SCH_A * neg_c + SCH_B  (per partition)
    mt_sb = None if CFG.get("par_reduce", True) else \
        sbuf.tile([1, 128], F32, tag="rcp")

    attn_sb = sbuf.tile([CQ, L], F32R, tag="vbig", name="attn_sb")
    rcp_bc = sbuf.tile([CQ, 1024], F32, tag="rbc")
    y_r = y.rearrange("(ko ki) l -> ki ko l", ki=128)
    y2_r = y2.rearrange("(ko ki) l -> ki ko l", ki=128)

    def scores_mms(s_ps, kb, c0, w):
        for off, pw in pieces(w):
            nc.tensor.matmul(
                s_ps[:, off : off + pw],
                k_sb[:, kb * 128 : (kb + 1) * 128],
                q_sb[:, c0 + off : c0 + off + pw],
                start=True,
                stop=True,
            )

    def attnv_mm(out_ps, et, kb, off, pw):
        nc.tensor.matmul(
            out_ps[0 : CQ + 1, off : off + pw],
            vt_sb[:, kb, :],
            et[:, 0:pw],
            start=(kb == 0),
            stop=(kb == NKB - 1),
        )

    def exp_tile(ci, u, s_ps, pw, n_units=NKB * 2):
        """exp(s - C) into a bf16 et tile, on the engine exp_engine says."""
        et = sbuf.tile([128, 512], BF16, tag="et", bufs=ET_BUFS,
                       name=f"et_{ci}_{u}")[:, 0:pw]
        eng = exp_engine(ci, u, n_units)
        if eng == 'a':
            nc.scalar.activation(et, s_ps, AF.Exp, bias=neg_c, scale=1.0)
        else:
            nc.vector.tensor_scalar(et.bitcast(U16), s_ps, SCH_A, b_eff,
                                    OP.mult, OP.add)
        return et

    def normalize(ci, out_ps):
        #   attn[:, c0:c0+w] = out_ps[0:96] * (1 / out_ps[96])
        # Mid chunks read straight from PSUM (no staging copy): the
        # accumulator's banks are only needed again two chunks later, and
        # skipping the copy cuts the serial recip -> broadcast -> mul latency
        # at each chunk boundary. Chunk 0 lives in the ps_proj pool whose
        # close barrier gates all of chunk 1, so it stages through SBUF with
        # one fast ScalarE copy to release its banks immediately.
        c0, w = CHUNKS[ci]
        if ci == len(CHUNKS) - 1:
            # tail: DVE is otherwise idle; straight from PSUM is the
            # shortest chain before phase4
            rcp_sb = sbuf.tile([1, 1024], F32, tag="rcp",
                               name=f"rcp_{ci}")[:, 0:w]
            nc.vector.reciprocal(rcp_sb, out_ps[CQ : CQ + 1, 0:w])
            nc.gpsimd.partition_broadcast(rcp_bc[:, 0:w], rcp_sb)
            nc.vector.tensor_mul(attn_sb[:, c0 : c0 + w], out_ps[0:CQ, 0:w],
                                 rcp_bc[:, 0:w])
        else:
            # stage through SBUF (ScalarE) and multiply on Pool: DVE only
            # pays the reciprocal, so its exp share starts ~2us earlier at
            # each chunk boundary — where ScalarE alone can't keep the
            # score slots draining at the PE's pace. Also frees the
            # accumulator's banks early (obig has a single slot).
            # the denominator row rides in the staged copy, so the
            # reciprocal reads SBUF — for chunk 0 that takes it off the
            # ps_proj pool-close path (which gates all of chunk 1)
            ostage = sbuf.tile([CQ + 1, 1024], F32, tag="ostage", bufs=1,
                               name=f"ostage_{ci}")[:, 0:w]
            nc.scalar.copy(ostage, out_ps[0 : CQ + 1, 0:w])
            rcp_sb = sbuf.tile([1, 1024], F32, tag="rcp",
                               name=f"rcp_{ci}")[:, 0:w]
            nc.vector.reciprocal(rcp_sb, ostage[CQ : CQ + 1, :])
            nc.gpsimd.partition_broadcast(rcp_bc[:, 0:w], rcp_sb)
            nc.gpsimd.tensor_mul(attn_sb[:, c0 : c0 + w], ostage[0:CQ, :],
                                 rcp_bc[:, 0:w])

    norm_tiles = {}

    def normalize_half(ci, out_ps, h):
        # Mid-chunk normalize, one 512-wide half at a time. Half 0 is
        # issued at the chunk's end; half 1 from inside the NEXT chunk's
        # k loop, a few units in: both halves are ready at the boundary,
        # so only priority (issue order) decides what the list scheduler
        # hoists into the boundary — deferring half 1 puts the next
        # chunk's first exps ahead of it in the ready-queue tie-breaks,
        # halving the boundary-resident ostage/recip work.
        c0, w = CHUNKS[ci]
        hw2 = w // 2
        hs = slice(h * hw2, (h + 1) * hw2)
        if h == 0:
            norm_tiles[ci] = (
                sbuf.tile([CQ + 1, 1024], F32, tag="ostage", bufs=1,
                          name=f"ostage_{ci}"),
                sbuf.tile([1, 1024], F32, tag="rcp", name=f"rcp_{ci}"),
            )
        ostage, rcp_sb = norm_tiles[ci]
        nc.scalar.copy(ostage[:, hs], out_ps[0 : CQ + 1, hs])
        nc.vector.reciprocal(rcp_sb[:, hs], ostage[CQ : CQ + 1, hs])
        nc.gpsimd.partition_broadcast(rcp_bc[:, hs], rcp_sb[:, hs])
        nc.gpsimd.tensor_mul(attn_sb[:, c0 + h * hw2 : c0 + (h + 1) * hw2],
                             ostage[0:CQ, hs], rcp_bc[:, hs])

    def phase4_unit(ps_pool, ci, oc, spread=True):
        # final projection + residual for one 128-row output chunk
        c0, w = CHUNKS[ci]
        if spread:
            # z halves borrow score slots (exp lookahead briefly 6 -> 4);
            # each half's add fires as soon as its matmul lands, DVE taking
            # one half and Pool the other
            y_sb = sbuf.tile([128, 1024], F32, tag="y", bufs=3,
                             name=f"y_sb_{ci}_{oc}")[:, 0:w]
            for g, (off, pw) in enumerate(pieces(w)):
                z_ps = ps_pool.tile([128, 512], F32, tag="s", bufs=S_BUFS,
                                    name=f"z_ps_{ci}_{oc}_{g}")[:, 0:pw]
                nc.tensor.matmul(
                    z_ps,
                    wl_sb[:, oc * 128 : (oc + 1) * 128],
                    attn_sb[:, c0 + off : c0 + off + pw],
                    start=True,
                    stop=True,
                )
                nc.vector.tensor_add(y_sb[:, off : off + pw], z_ps,
                               x_sb[:, oc, c0 + off : c0 + off + pw].bitcast(F32))
            nc.sync.dma_start(out=y_r[:, oc, slice(c0, c0 + w)], in_=y_sb)
            return
        # late-issued unit: residual via identity-matmul accumulate and a
        # ScalarE copy — its matmuls fill the last chunk's normalize-wait
        # PE gap, and it keeps the ~1.2us DVE add out of the tail window
        y_sb = sbuf.tile([128, 1024], F32, tag="y", bufs=3,
                         name=f"y_sb_{ci}_{oc}")[:, 0:w]
        for g, (off, pw) in enumerate(pieces(w)):
            zp = ps_pool.tile([128, 512], F32, tag="s", bufs=S_BUFS,
                              name=f"z_late_{ci}_{oc}_{g}")[:, 0:pw]
            nc.tensor.matmul(zp, ident_r, x_sb[:, oc, c0 + off : c0 + off + pw],
                             start=True, stop=False)
            nc.tensor.matmul(
                zp,
                wl_sb[:, oc * 128 : (oc + 1) * 128],
                attn_sb[:, c0 + off : c0 + off + pw],
                start=False,
                stop=True,
            )
            nc.scalar.copy(y_sb[:, off : off + pw], zp)
        nc.sync.dma_start(out=y_r[:, oc, slice(c0, c0 + w)], in_=y_sb)

    def phase4_last(ps_pool, ci, out_ps):
        # Last chunk: normalize from PSUM in two 256-wide halves (both
        # reciprocals on DVE up front, broadcasts on Pool, then the muls)
        # so the first projection matmuls start ~1us earlier than a
        # full-width chain; residual via identity-matmul accumulates
        # issued during the normalize latency; PSUM->SBUF bf16 copies
        # split 3 ScalarE / 3 DVE feeding three paired DMAs on SP (each
        # group one ScalarE + one DVE copy, so the last DMA issues as
        # early as either engine allows).
        c0, w = CHUNKS[ci]
        hw = w // 2
        zs = []
        for oc in range(KC):
            z_ps = ps_pool.tile([128, 512], F32, tag="s", bufs=S_BUFS,
                                name=f"z_ps_{ci}_{oc}")[:, 0:w]
            nc.tensor.matmul(
                z_ps,
                ident_r,
                x_sb[:, oc, c0 : c0 + w],
                start=True,
                stop=False,
            )
            zs.append(z_ps)
        # one rcp tile sliced per half: separate ring allocations would
        # serialize recip1 behind bcast0's read of the single-buf slot
        rcp2 = sbuf.tile([1, 1024], F32, tag="rcp", name=f"rcp_{ci}")
        for h in range(2):
            hs = slice(h * hw, (h + 1) * hw)
            nc.vector.reciprocal(rcp2[:, hs], out_ps[CQ : CQ + 1, hs])
            nc.gpsimd.partition_broadcast(rcp_bc[:, hs], rcp2[:, hs])
        for h in range(2):
            hs = slice(h * hw, (h + 1) * hw)
            nc.vector.tensor_mul(attn_sb[:, c0 + h * hw : c0 + (h + 1) * hw],
                                 out_ps[0:CQ, hs], rcp_bc[:, hs])
            for oc in range(KC):
                nc.tensor.matmul(
                    zs[oc][:, hs],
                    wl_sb[:, oc * 128 : (oc + 1) * 128],
                    attn_sb[:, c0 + h * hw : c0 + (h + 1) * hw],
                    start=False,
                    stop=(h == 1),
                )
        ysg = [sbuf.tile([128, 2, 512], BF16, tag=f"ylast{g}", bufs=1,
                         name=f"y_last_{g}") for g in range(3)]
        for g in range(3):
            nc.scalar.copy(ysg[g][:, 0, :], zs[2 * g])
            nc.vector.tensor_copy(ysg[g][:, 1, :], zs[2 * g + 1])
        # SP queue only: a ScalarE-issued DMA would block ScalarE's in-order
        # queue and delay the remaining copies
        for g in range(3):
            nc.sync.dma_start(out=y2_r[:, 2 * g : 2 * g + 2, :], in_=ysg[g])

    # ---- phase 1 + attention chunk 0 (512 wide), interleaved ------------
    # projections run in 512-column groups; as each group's K/V land, the
    # corresponding k-blocks of chunk 0 are scored/exp'd/accumulated.
    with (
        tc.tile_pool(name="ps_proj", bufs=1, space="PSUM") as ps_proj,
        tc.tile_pool(name="ps_aux", bufs=2, space="PSUM") as ps_aux,
    ):
        out0_ps = ps_proj.tile([128, 512], F32, tag="o0", name="out0_ps")
        # PE p-state warmup: the clock runs at half rate until 3us of
        # CONTINUOUS busy, and the first x piece only lands ~3.9us in.
        # Dummy identity transposes (no readers, recycled ps_aux ring)
        # keep the PE busy from ~0.3us so the ramp completes before the
        # first projection matmul — which then runs at the full 2.4GHz.
        for wu in range(CFG.get("warmup", 0)):
            wu_ps = ps_aux.tile([128, 128], F32, tag="sm", name=f"wu_{wu}")
            nc.tensor.transpose(wu_ps, ident, ident)
        pend_attnv = []  # attnV lag FIFO so PE never waits on exp in-order
        for gp in range(8):
            gs = slice(gp * 512, (gp + 1) * 512)
            tiles = [
                ps_proj.tile([CQ, 512], F32, tag=f"proj{t}", name=f"p_ps_{t}_{gp}")
                for t in range(3)
            ]
            for kc in range(KC):
                for t in range(3):
                    if gp == 0 and kc == 0 and CFG.get("split_start", True):
                        # x arrives in two 256-col halves; start on the first
                        for ho in (0, 256):
                            nc.tensor.matmul(
                                tiles[t][:, ho : ho + 256],
                                w_sb[:, 0, t * CQ : (t + 1) * CQ],
                                x_sb[:, 0, ho : ho + 256],
                                start=True,
                                stop=False,
                                skip_group_check=True,
                            )
                        continue
                    nc.tensor.matmul(
                        tiles[t],
                        w_sb[:, kc, t * CQ : (t + 1) * CQ],
                        x_sb[:, kc, gs],
                        start=(kc == 0),
                        stop=(kc == KC - 1),
                        skip_group_check=(gp == 0),
                    )
            for t, dst in ((0, q_sb), (1, k_sb), (2, v_sb)):
                if t == 1:
                    nc.vector.tensor_copy(dst[:, gs], tiles[t])
                else:
                    nc.scalar.copy(dst[:, gs], tiles[t])

            # chunk-0 attention for this group's 4 k-blocks
            for kb in range(4 * gp, 4 * gp + 4):
                s_ps = ps_proj.tile([128, 512], F32, tag="s0", bufs=2,
                                    name=f"s_ps_0_{kb}")
                scores_mms(s_ps, kb, 0, 512)
                if kb == 0:
                    # shift estimate from these 32k scores (statistically
                    # ample for a shift that merely has to land within
                    # ~+-80 of the true max). This chain gates every
                    # chunk-0 exp, so it must be SHORT.
                    nc.vector.reduce_max(m_row, s_ps[:, 0:CFG.get('est_w', 512)], axis=AX.X)
                    if CFG.get("par_reduce", True):
                        # one Pool all-reduce replaces the four-link
                        # transpose/copy/reduce/broadcast partition chain
                        nc.gpsimd.partition_all_reduce(
                            gmax_bc, m_row, 128, bass_isa.ReduceOp.max)
                    else:
                        mt_ps = ps_aux.tile([1, 128], F32, tag="sm")
                        nc.tensor.transpose(mt_ps, m_row, ident)
                        nc.vector.tensor_copy(mt_sb[:, 0:128], mt_ps)
                        nc.vector.reduce_max(small[0:1, 11:12],
                                             mt_sb[:, 0:128], axis=AX.X)
                        nc.gpsimd.partition_broadcast(gmax_bc,
                                                      small[0:1, 11:12])
                    # neg_c = -(gmax + MARGIN)
                    nc.scalar.activation(neg_c, gmax_bc, AF.Copy,
                                         bias=-MARGIN, scale=-1.0)
                    # b_eff = SCH_A * neg_c + SCH_B (for the approx engines;
                    # computing it directly from gmax in parallel with the
                    # neg_c activation simmed neutral — this link has slack)
                    nc.vector.tensor_scalar(b_eff, neg_c, SCH_A, SCH_B,
                                            OP.mult, OP.add)
                et = exp_tile(0, kb, s_ps, 512, n_units=NKB)
                if len(pend_attnv) >= attnv_lag(0):
                    attnv_mm(out0_ps, *pend_attnv.pop(0))
                pend_attnv.append((et, kb, 0, 512))
            # V -> V^T transposes for this group's 4 l-blocks (the last
            # group's copies optionally on ScalarE so DVE is free for the
            # bridge exps that gate the pool swap)
            for lb in range(4 * gp, 4 * gp + 4):
                t_ps = ps_aux.tile([128, CQ], BF16, tag="sm", name=f"t_ps_{lb}")
                nc.tensor.transpose(
                    t_ps, v_sb[:, lb * 128 : (lb + 1) * 128], ident_bf[0:CQ, 0:CQ]
                )
                if gp == 7 and CFG.get("vt7_scalar"):
                    nc.scalar.copy(vt_sb[:, lb, 0:CQ], t_ps)
                else:
                    nc.vector.tensor_copy(vt_sb[:, lb, 0:CQ], t_ps)

        for pa in pend_attnv:
            attnv_mm(out0_ps, *pa)
        # chunk-0 normalize first: its ScalarE staging copy releases the
        # out0 banks so the pool-close barrier (gating all of chunk 1) isn't
        # stuck behind the bridge exps
        normalize(0, out0_ps)
        # bridge: score+exp chunk-1's k-block 0 in this pool's slots so
        # ScalarE never idles across the PSUM pool swap
        bridge_units = []
        for bu in range(2 * CFG.get("bridge_kb", 1)):
            kb, h = bu // 2, bu % 2
            sb_ps = ps_proj.tile([128, 512], F32, tag="s0", bufs=2,
                                 name=f"sb_ps_{bu}")
            nc.tensor.matmul(
                sb_ps, k_sb[:, kb * 128 : (kb + 1) * 128],
                q_sb[:, 512 + h * 512 : 512 + (h + 1) * 512],
                start=True, stop=True,
            )
            bet = sbuf.tile([128, 512], BF16, tag="et", bufs=ET_BUFS,
                            name=f"et_1_0_{bu}")
            # engine choice: the pool close (gating all of chunk 1) waits on
            # these exps' PSUM reads, racing the other engine's backlog
            if CFG.get("bridge_eng", "vv")[bu % len(CFG.get("bridge_eng", "vv"))] == 'v':
                nc.vector.tensor_scalar(bet.bitcast(U16), sb_ps, SCH_A, b_eff,
                                        OP.mult, OP.add)
            else:
                nc.scalar.activation(bet, sb_ps, AF.Exp, bias=neg_c, scale=1.0)
            bridge_units.append((bet, kb, h * 512, 512))

    # ---- attention chunks 1..4 ------------------------------------------
    with tc.tile_pool(name="ps_attn", bufs=1, space="PSUM") as ps_attn:
        prev_ps = [None]   # previous chunk's accumulator awaiting half-1
        for ci in range(1, len(CHUNKS)):
            c0, w = CHUNKS[ci]
            out_ps = ps_attn.tile(
                [128, 1024], F32, tag="obig", bufs=1, name=f"out_ps_{ci}"
            )
            # attnV trails exp by ATTNV_LAG 512-wide units; 4 s_ps slots let
            # up to 4 exps run concurrently across ScalarE/DVE/Pool
            pend = list(bridge_units) if ci == 1 else []
            nh = len(pieces(w))
            for kb in range(CFG.get("bridge_kb", 1) if ci == 1 else 0, NKB):
                for h, (off, pw) in enumerate(pieces(w)):
                    u = kb * nh + h
                    s_ps = ps_attn.tile(
                        [128, 512], F32, tag="s", bufs=S_BUFS,
                        name=f"s_ps_{ci}_{u}"
                    )[:, 0:pw]
                    nc.tensor.matmul(
                        s_ps,
                        k_sb[:, kb * 128 : (kb + 1) * 128],
                        q_sb[:, c0 + off : c0 + off + pw],
                        start=True,
                        stop=True,
                    )
                    et = exp_tile(ci, u, s_ps, pw, n_units=NKB * nh)
                    if len(pend) >= attnv_lag(ci):
                        attnv_mm(out_ps, *pend.pop(0))
                    pend.append((et, kb, off, pw))
                    if u == CFG.get("h1_at", 3) and ci >= 2 and \
                            CFG.get("split_norm", False):
                        normalize_half(ci - 1, prev_ps[0], 1)
                # spread the previous chunk's phase 4 through this chunk's
                # k loop, starting at kb=8 so the previous chunk's normalize
                # chain (which the z matmuls depend on) has finished — PE is
                # in-order, so an early-enqueued z matmul would stall scores
                last = ci == len(CHUNKS) - 1
                sp0, step = CFG.get("sp_last", (16, 3)) if last else CFG.get("sp_mid", (12, 3))
                # all spread units must fit inside this chunk's kb range — a
                # unit past kb=31 would silently drop an output block
                nsp = KC - 1 if (last and CFG.get("late6", True)) else KC
                assert sp0 + step * (nsp - 1) < NKB
                if (kb - sp0) % step == 0 and sp0 <= kb < sp0 + step * nsp:
                    phase4_unit(ps_attn, ci - 1, (kb - sp0) // step)
            for pe in pend:
                attnv_mm(out_ps, *pe)
            if ci == len(CHUNKS) - 1 and CFG.get("late6", True):
                # chunk-3's last output block, issued after the drain: its
                # matmuls fill the PE gap while the tail normalize runs
                phase4_unit(ps_attn, ci - 1, KC - 1, spread=False)
            if ci < len(CHUNKS) - 1:
                if CFG.get("split_norm", False):
                    normalize_half(ci, out_ps, 0)
                    prev_ps[0] = out_ps
                else:
                    normalize(ci, out_ps)

        # last chunk's normalize halves + phase 4
        phase4_last(ps_attn, len(CHUNKS) - 1, out_ps)


def build(loop_iters=1):
    nc = bacc.Bacc("TRN2", target_bir_lowering=False, debug=False, num_devices=8)
    x = nc.dram_tensor("x", [C, L], F32, kind="ExternalInput").ap()
    wqkv_t = nc.dram_tensor("wqkv_t", [C, 3 * CQ], F32, kind="ExternalInput").ap()
    wl_t = nc.dram_tensor("wl_t", [CQ, C], F32, kind="ExternalInput").ap()
    y = nc.dram_tensor("y", [C, L], F32, kind="ExternalOutput").ap()
    # last q-chunk's output in bf16: halves the tail's DMA wire time; the
    # 0.39% bf16 step is well under the error budget (host converts back)
    y2 = nc.dram_tensor("y2", [C, CHUNKS[-1][1]], mybir.dt.bfloat16,
                        kind="ExternalOutput").ap()

    with tile.TileContext(nc) as tc:
        with tc.tile_pool(name="sbuf", bufs=1) as sbuf:
            if loop_iters > 1:
                engines = (
                    mybir.EngineType.PE,
                    mybir.EngineType.Activation,
                    mybir.EngineType.DVE,
                    mybir.EngineType.Pool,
                    mybir.EngineType.SP,
                )
                with tc.For_i(0, loop_iters, hint_engines=engines):
                    body(nc, tc, sbuf, x, wqkv_t, wl_t, y, y2)
            else:
                body(nc, tc, sbuf, x, wqkv_t, wl_t, y, y2)

    nc.compile()
    return nc


_cached_nc = None


def kernel(x, Wq, Wk, Wv, Wlast, gamma):
    global _cached_nc
    x = np.ascontiguousarray(np.asarray(x, dtype=np.float32))
    B = x.shape[0]
    assert B == 8 and x.shape[1:] == (C, 64, 64)
    wqkv_t = np.ascontiguousarray(
        np.concatenate([Wq, Wk, Wv], axis=0).T.astype(np.float32)
    )
    wl_t = np.ascontiguousarray(
        (np.asarray(Wlast, np.float32) * np.float32(np.asarray(gamma)[0])).T
    )

    if _cached_nc is None:
        _cached_nc = build()
    nc = _cached_nc

    in_maps = [
        {
            "x": np.ascontiguousarray(x[b].reshape(C, L)),
            "wqkv_t": wqkv_t,
            "wl_t": wl_t,
        }
        for b in range(B)
    ]
    res = bass_utils.run_bass_kernel_spmd(nc, in_maps, core_ids=list(range(B)))
    lw = CHUNKS[-1][1]
    outs = []
    for b in range(B):
        yb = np.array(res.results[b]["y"]).reshape(C, L)
        yb[:, L - lw:] = res.results[b]["y2"].astype(np.float32)
        outs.append(yb.reshape(C, 64, 64))
    return np.stack(outs).astype(np.float32)

